# revision 30
# baseline (speedup 1.0000x reference)
"""Trainium2 Bass kernel: block-match SAD cost volume + spiral-tie-break argmin.

Problem (nn_CalculateVector): inputs w1, w2 [1,270,480,50,16] f16 (integer
values 0..255).  Per pixel: SAD cost over K2=16 for 50 candidates, argmin
over the 49 block-match candidates with center-out-spiral tie-break, then
input-MV override, LUT vector output, and template gather.

Strategy: fully data-parallel over the 129600 pixels, 16200 per NeuronCore.
Within a core, pixels are laid out partition-major (partition p owns 126
consecutive pixels) and processed in chunks of TCH pixels per partition.

The spiral argmin is computed exactly with a fused key: key[c] = cost[c] +
RANK[c]/64 (RANK = spiral rank, unique per candidate; cost is an integer
<= 4080 so the fp32 key is exact and min(key) implements first-occurrence-
in-spiral-order argmin).  The input-MV candidate gets rank 63 which makes
min/compare against it implement the reference's strict `<` semantics.

Gathers (VEC_LUT[argmin], w1[argmin]) are computed as one-hot dot products
on the vector engine (eq = key == minkey is exactly one-hot since keys are
unique within a pixel).
"""

import numpy as np

import concourse.bass as bass
import concourse.tile as tile
from concourse import bacc, mybir
from concourse.bass_utils import run_bass_kernel_spmd

SR = 3
NSIDE = 2 * SR + 1
N2 = NSIDE * NSIDE           # 49
K2 = 16
NCAND = N2 + 1               # 50
E = NCAND * K2               # 800 elements per pixel
B, H, W = 1, 270, 480
NPIX = B * H * W             # 129600
NCORES = 8
PPC = NPIX // NCORES         # 16200
TPART = PPC // 128           # 126
MAIN = TPART * 128           # 16128
TAIL_OFF = PPC - 128         # 16072 (tail tile overlaps main region; same values)
TCH = 6                      # pixels per partition per chunk
NCHUNK = TPART // TCH        # 21
G = 3                        # chunks per batched scalar-chain group

F16 = mybir.dt.float16
F32 = mybir.dt.float32
I32 = mybir.dt.int32
U8 = mybir.dt.uint8
Alu = mybir.AluOpType
AX = mybir.AxisListType


def _spiral_order(sz):
    n = 2 * sz + 1
    i = j = 0
    order = [(j + sz) * n + (i + sz)]
    dirs = [(1, 0), (0, 1), (-1, 0), (0, -1)]
    d, step = 0, 1
    while len(order) < n * n:
        for _ in range(2):
            di, dj = dirs[d]
            for _ in range(step):
                i += di
                j += dj
                if abs(i) <= sz and abs(j) <= sz:
                    order.append((j + sz) * n + (i + sz))
            d = (d + 1) % 4
        step += 1
    return np.asarray(order, dtype=np.int32)


def _vec_lut(sz):
    rng = np.arange(-sz, sz + 1)
    jj, ii = np.meshgrid(rng, rng, indexing="ij")
    return (-1.0 * np.stack([jj, ii], axis=-1).reshape(-1, 2)).astype(np.float16)


SPIRAL = _spiral_order(SR)                       # [49]
RANK = np.empty(N2, np.int64)
RANK[SPIRAL] = np.arange(N2)                     # inverse permutation
VEC = _vec_lut(SR)                               # [49, 2] f16, original order
RANK64 = np.zeros(NCAND, np.float32)
RANK64[:N2] = RANK / 64.0
RANK64[N2] = 63.0 / 64.0
VEC_DC = np.ascontiguousarray(VEC.T)             # [2, 49] (d, c) layout

# v2 encoding: key[c] = cost[c] + (RANK[c]*64 + c)/4096 (exact in fp32;
# ordering by RANK dominates so min() still tie-breaks by spiral rank, and
# frac*4096 = RANK*64 + c recovers the winning candidate).  MV candidate gets
# frac 4032/4096 = 63*64/4096 which preserves the strict-< mask semantics.
NRANKC = N2 * 64 + 64                            # 3200 table entries (max idx 3120)
RKEY2 = np.zeros(NCAND, np.float32)
RKEY2[:N2] = (RANK * 64 + np.arange(N2)) / 4096.0
RKEY2[N2] = 4032.0 / 4096.0
VEC_RANKC = np.zeros((NRANKC, 2), np.float16)    # rankc -> VEC[c]
for _c in range(N2):
    VEC_RANKC[RANK[_c] * 64 + _c] = VEC[_c]
IMASK2 = np.zeros((128, 16 * 2), np.float16)     # diag extract: i == p%16
for _p in range(128):
    IMASK2[_p, (_p % 16) * 2 : (_p % 16) * 2 + 2] = 1.0
IMASKK = np.zeros((128, 16 * K2), np.float16)    # same at K2 granularity
for _p in range(128):
    IMASKK[_p, (_p % 16) * K2 : (_p % 16 + 1) * K2] = 1.0
OFFS50 = np.tile(np.arange(TCH, dtype=np.float32) * NCAND, G)  # + t*50 per chunk


I16 = mybir.dt.int16


def build_program(nc, sim_safe=False):
    w1 = nc.dram_tensor("w1", [PPC, E], F16, kind="ExternalInput").ap()
    w2 = nc.dram_tensor("w2", [PPC, E], F16, kind="ExternalInput").ap()
    rank_d = nc.dram_tensor("rank50", [128, NCAND], F32, kind="ExternalInput").ap()
    vec_d = nc.dram_tensor("vecdc", [128, 2 * N2], F16, kind="ExternalInput").ap()
    rkey_d = nc.dram_tensor("rkey2", [128, NCAND], F32, kind="ExternalInput").ap()
    vecrk_d = nc.dram_tensor(
        "vecrk", [128, NRANKC * 2], F16, kind="ExternalInput"
    ).ap()
    imask_d = nc.dram_tensor("imask2", [128, 32], F16, kind="ExternalInput").ap()
    imaskk_d = nc.dram_tensor(
        "imaskk", [128, 16 * K2], F16, kind="ExternalInput"
    ).ap()
    offs_d = nc.dram_tensor(
        "offs50", [128, G * TCH], F32, kind="ExternalInput"
    ).ap()
    o_vec = nc.dram_tensor("o_vec", [PPC, 2], F16, kind="ExternalOutput").ap()
    o_tpl = nc.dram_tensor("o_tpl", [PPC, K2], F16, kind="ExternalOutput").ap()
    o_msk = nc.dram_tensor("o_msk", [PPC, 1], U8, kind="ExternalOutput").ap()
    o_mcv = nc.dram_tensor("o_mcv", [PPC, 1], I32, kind="ExternalOutput").ap()

    from contextlib import ExitStack

    with tile.TileContext(nc) as tc, ExitStack() as ctx:
        cpool = ctx.enter_context(tc.tile_pool(name="const", bufs=1))
        w1p = ctx.enter_context(
            tc.tile_pool(name="w1p", bufs=(2 * G + 1))
        )
        w2p = ctx.enter_context(tc.tile_pool(name="w2p", bufs=2))
        wp = ctx.enter_context(tc.tile_pool(name="work", bufs=2))
        sp = ctx.enter_context(tc.tile_pool(name="stage", bufs=1))

        rank_t = cpool.tile([128, NCAND], F32, tag="rank_t")
        nc.sync.dma_start(rank_t[:], rank_d)
        vec_t = cpool.tile([128, 2 * N2], F16, tag="vec_t")
        nc.sync.dma_start(vec_t[:], vec_d)
        rkey_t = cpool.tile([128, NCAND], F32, tag="rkey_t")
        nc.sync.dma_start(rkey_t[:], rkey_d)
        vecrk_t = cpool.tile([128, NRANKC * 2], F16, tag="vecrk_t")
        nc.sync.dma_start(vecrk_t[:], vecrk_d)
        imask_t = cpool.tile([128, 32], F16, tag="imask_t")
        nc.sync.dma_start(imask_t[:], imask_d)
        offs_t = cpool.tile([128, TCH], F32, tag="offs_t")
        nc.sync.dma_start(offs_t[:], offs_d)

        st_vec = sp.tile([128, TPART * 2], F16, tag="st_vec")
        st_msk = sp.tile([128, TPART], U8, tag="st_msk")
        st_mcv = sp.tile([128, TPART], I32, tag="st_mcv")
        # gathered templates, (chunk, t, i, k) per partition; valid at i==p%16
        st_tplg = sp.tile([128, NCHUNK * TCH * 16 * K2], F16, tag="st_tplg")

        w1m = w1[0:MAIN, :].rearrange("(p q) e -> p (q e)", p=128)
        w2m = w2[0:MAIN, :].rearrange("(p q) e -> p (q e)", p=128)

        def chunk(T, src1, src2, ovec, otpl, omsk, omcv):
            # v1 path (one-hot dot products on DVE) — used for the tail only.
            w1t = w1p.tile([128, T * E], F16, tag="w1t")
            nc.sync.dma_start(w1t[:], src1)
            w2t = w2p.tile([128, T * E], F16, tag="w2t")
            nc.sync.dma_start(w2t[:], src2)

            diff = wp.tile([128, T * E], F16, tag="diff_v1")
            nc.vector.tensor_tensor(diff[:], w1t[:], w2t[:], op=Alu.subtract)

            cost = wp.tile([128, T * NCAND], F32, tag="cost_v1")
            cost3 = cost[:].rearrange("p (t c) -> p t c", c=NCAND)
            nc.vector.tensor_reduce(
                cost3,
                diff[:].rearrange("p (t c k) -> p t c k", c=NCAND, k=K2),
                axis=AX.X,
                op=Alu.add,
                apply_absolute_value=True,
            )

            key = wp.tile([128, T * NCAND], F32, tag="key_v1")
            key3 = key[:].rearrange("p (t c) -> p t c", c=NCAND)
            nc.vector.tensor_tensor(
                key3,
                cost3,
                rank_t[:].unsqueeze(1).broadcast_to([128, T, NCAND]),
                op=Alu.add,
            )

            minkbm = wp.tile([128, T], F32, tag="minkbm_v1")
            nc.vector.tensor_reduce(
                minkbm[:], key3[:, :, 0:N2], axis=AX.X, op=Alu.min
            )

            kmv = key3[:, :, N2:NCAND].rearrange("p t one -> p (t one)")
            mk50 = wp.tile([128, T], F32, tag="mk50_v1")
            nc.vector.tensor_tensor(mk50[:], kmv, minkbm[:], op=Alu.min)

            maskf = wp.tile([128, T], F32, tag="maskf_v1")
            nc.vector.tensor_tensor(maskf[:], kmv, minkbm[:], op=Alu.is_lt)
            nc.scalar.copy(omsk, maskf[:])

            nc.vector.tensor_reduce(omcv, cost3, axis=AX.X, op=Alu.min)

            keyct = key[:].rearrange("p (t c) -> p c t", c=NCAND)
            eq50 = wp.tile([128, NCAND * T], F16, tag="eq50_v1")
            eq50v = eq50[:].rearrange("p (c t) -> p c t", t=T)
            nc.vector.tensor_tensor(
                eq50v,
                keyct,
                mk50[:].unsqueeze(1).broadcast_to([128, NCAND, T]),
                op=Alu.is_equal,
            )
            eq49 = wp.tile([128, N2 * T], F16, tag="eq49_v1")
            eq49v = eq49[:].rearrange("p (c t) -> p c t", t=T)
            nc.vector.tensor_tensor(
                eq49v,
                keyct[:, 0:N2, :],
                minkbm[:].unsqueeze(1).broadcast_to([128, N2, T]),
                op=Alu.is_equal,
            )

            prodv = wp.tile([128, T * 2 * N2], F16, tag="prodv_v1")
            prodv4 = prodv[:].rearrange("p (t d c) -> p t d c", d=2, c=N2)
            e49_tdc = (
                eq49[:]
                .rearrange("p (c t) -> p t c", t=T)
                .unsqueeze(2)
                .broadcast_to([128, T, 2, N2])
            )
            vdc = (
                vec_t[:]
                .rearrange("p (d c) -> p d c", c=N2)
                .unsqueeze(1)
                .broadcast_to([128, T, 2, N2])
            )
            nc.vector.tensor_tensor(prodv4, e49_tdc, vdc, op=Alu.mult)
            with nc.allow_low_precision("one-hot dot, sums are exact"):
                nc.vector.tensor_reduce(ovec, prodv4, axis=AX.X, op=Alu.add)

            prod8 = wp.tile([128, T * K2 * NCAND], F16, tag="prod8_v1")
            prod84 = prod8[:].rearrange("p (t k c) -> p t k c", k=K2, c=NCAND)
            w1_tkc = w1t[:].rearrange("p (t c k) -> p t k c", c=NCAND, k=K2)
            eq50_tkc = (
                eq50[:]
                .rearrange("p (c t) -> p t c", t=T)
                .unsqueeze(2)
                .broadcast_to([128, T, K2, NCAND])
            )
            nc.vector.tensor_tensor(prod84, w1_tkc, eq50_tkc, op=Alu.mult)
            with nc.allow_low_precision("one-hot dot, sums are exact"):
                nc.vector.tensor_reduce(otpl, prod84, axis=AX.X, op=Alu.add)

        # ---- v3 main path: per-chunk streaming (sub+reduce) with the scalar
        # chain batched per group of G chunks, GPSIMD doing gathers and the
        # small tensor-tensor chain, and partition-strided extraction DMAs.
        def part_load(cis):
            # loads + subtract + grouped SAD reduce for a group of chunks
            w1ts = []
            cost_w = wp.tile([128, G * TCH * NCAND], F32, tag="cost_w")
            for l, ci in enumerate(cis):
                t0 = ci * TCH
                sl = slice(t0 * E, (t0 + TCH) * E)
                w1t = w1p.tile([128, TCH * E], F16, tag="w1t")
                nc.sync.dma_start(w1t[:], w1m[:, sl])
                w2t = w2p.tile([128, TCH * E], F16, tag="w2t")
                nc.sync.dma_start(w2t[:], w2m[:, sl])
                w1ts.append(w1t)
                # diff in place of w2t (w2 is dead after this)
                nc.vector.tensor_tensor(w2t[:], w1t[:], w2t[:], op=Alu.subtract)
                nc.vector.tensor_reduce(
                    cost_w[:, l * TCH * NCAND : (l + 1) * TCH * NCAND].rearrange(
                        "p (t c) -> p t c", c=NCAND
                    ),
                    w2t[:].rearrange("p (t c k) -> p t c k", c=NCAND, k=K2),
                    axis=AX.X,
                    op=Alu.add,
                    apply_absolute_value=True,
                )
            return {"cis": cis, "w1ts": w1ts, "cost_w": cost_w}

        def part_chain(st):
            cis, w1ts, cost_w = st["cis"], st["w1ts"], st["cost_w"]
            W = len(cis) * TCH
            cost3 = cost_w[:, : W * NCAND].rearrange("p (t c) -> p t c", c=NCAND)
            key_w = wp.tile([128, G * TCH * NCAND], F32, tag="key_w")
            key3 = key_w[:, : W * NCAND].rearrange("p (t c) -> p t c", c=NCAND)
            nc.vector.tensor_tensor(
                key3,
                cost3,
                rkey_t[:].unsqueeze(1).broadcast_to([128, W, NCAND]),
                op=Alu.add,
            )
            minkbm = wp.tile([128, G * TCH], F32, tag="minkbm")
            nc.vector.tensor_reduce(
                minkbm[:, :W], key3[:, :, 0:N2], axis=AX.X, op=Alu.min
            )
            mincbm = wp.tile([128, G * TCH], F32, tag="mincbm")
            nc.vector.tensor_reduce(
                mincbm[:, :W], cost3[:, :, 0:N2], axis=AX.X, op=Alu.min
            )
            kmv = key3[:, :, N2:NCAND].rearrange("p t one -> p (t one)")
            cmv = cost3[:, :, N2:NCAND].rearrange("p t one -> p (t one)")
            maskf = wp.tile([128, G * TCH], F32, tag="maskf")
            nc.vector.tensor_tensor(maskf[:, :W], kmv, minkbm[:, :W], op=Alu.is_lt)
            p0 = cis[0] * TCH
            nc.scalar.copy(st_msk[:, p0 : p0 + W], maskf[:, :W])
            nc.vector.tensor_tensor(
                st_mcv[:, p0 : p0 + W], cmv, mincbm[:, :W], op=Alu.min
            )
            frac = wp.tile([128, G * TCH], F32, tag="frac")
            nc.gpsimd.tensor_tensor(
                frac[:, :W], minkbm[:, :W], mincbm[:, :W], op=Alu.subtract
            )
            rci = wp.tile([128, G * TCH], I16, tag="rci")
            nc.vector.tensor_scalar(
                rci[:, :W], frac[:, :W], 4096.0, None, op0=Alu.mult
            )
            gv = wp.tile([128, G * TCH * 32], F16, tag="gv")
            nc.gpsimd.ap_gather(
                gv[:, : W * 32],
                vecrk_t[:],
                rci[:, :W],
                channels=128,
                num_elems=NRANKC,
                d=2,
                num_idxs=16 * W,
            )
            vprod = wp.tile([128, G * TCH * 32], F16, tag="vprod")
            nc.gpsimd.tensor_tensor(
                vprod[:, : W * 32].rearrange("p (t i d) -> p t i d", i=16, d=2),
                gv[:, : W * 32].rearrange("p (t i d) -> p t i d", i=16, d=2),
                imask_t[:]
                .rearrange("p (i d) -> p i d", d=2)
                .unsqueeze(1)
                .broadcast_to([128, W, 16, 2]),
                op=Alu.mult,
            )
            ovec = st_vec[:, p0 * 2 : (p0 + W) * 2].rearrange(
                "p (t d) -> p t d", d=2
            )
            with nc.allow_low_precision("one-hot extract, sums exact"):
                nc.vector.tensor_reduce(
                    ovec,
                    vprod[:, : W * 32].rearrange("p (t i d) -> p t d i", i=16, d=2),
                    axis=AX.X,
                    op=Alu.add,
                )
            vsl = st_vec[:, p0 * 2 : (p0 + W) * 2].rearrange("p (t d) -> p t d", d=2)
            v0 = vsl[:, :, 0:1].rearrange("p t one -> p (t one)")
            v1 = vsl[:, :, 1:2].rearrange("p t one -> p (t one)")
            # s = 7*v0 + v1; c = 24 - s; 49 - c = 25 + s
            s_w = wp.tile([128, G * TCH], F32, tag="s_w")
            nc.vector.scalar_tensor_tensor(
                s_w[:, :W], v0, 7.0, v1, op0=Alu.mult, op1=Alu.add
            )
            cidx = wp.tile([128, G * TCH], F32, tag="cidx")
            nc.vector.tensor_scalar(
                cidx[:, :W], s_w[:, :W], -1.0, 24.0, op0=Alu.mult, op1=Alu.add
            )
            t1 = wp.tile([128, G * TCH], F32, tag="t1")
            nc.vector.tensor_scalar(
                t1[:, :W], s_w[:, :W], 1.0, 25.0, op0=Alu.mult, op1=Alu.add
            )
            nc.gpsimd.tensor_tensor(t1[:, :W], t1[:, :W], maskf[:, :W], op=Alu.mult)
            nc.gpsimd.tensor_tensor(t1[:, :W], t1[:, :W], cidx[:, :W], op=Alu.add)
            for l, ci in enumerate(cis):
                # per-chunk tidx tile: ap_gather needs an aligned index base
                tidx = wp.tile([128, TCH], I16, tag=f"tidx{l}")
                nc.vector.tensor_tensor(
                    tidx[:],
                    t1[:, l * TCH : (l + 1) * TCH],
                    offs_t[:, 0:TCH],
                    op=Alu.add,
                )
                nc.gpsimd.ap_gather(
                    st_tplg[:, ci * (TCH * 16 * K2) : (ci + 1) * (TCH * 16 * K2)],
                    w1ts[l][:],
                    tidx[:],
                    channels=128,
                    num_elems=TCH * NCAND,
                    d=K2,
                    num_idxs=16 * TCH,
                )
            if not sim_safe:
                tpl_extract_dmas(cis[0], cis[-1] + 1)

        def tpl_extract_dmas(c0, c1):
            # partition-strided extraction for chunks [c0, c1)
            o6 = o_tpl[0:MAIN, :].rearrange(
                "(q j c t) k -> q j c t k", j=16, c=NCHUNK, t=TCH
            )
            for j in range(16):
                sb = st_tplg[j::16].rearrange(
                    "q (c t i k) -> q c t i k", c=NCHUNK, t=TCH, i=16
                )[:, c0:c1, :, j : j + 1, :]
                nc.sync.dma_start(o6[:, j : j + 1, c0:c1], sb)

        # tail first so it overlaps the main stream: 128 pixels
        # [TAIL_OFF, PPC), one per partition
        tl_vec = sp.tile([128, 2], F16, tag="tl_vec")
        tl_tpl = sp.tile([128, K2], F16, tag="tl_tpl")
        tl_msk = sp.tile([128, 1], U8, tag="tl_msk")
        tl_mcv = sp.tile([128, 1], I32, tag="tl_mcv")
        chunk(
            1,
            w1[TAIL_OFF:PPC, :],
            w2[TAIL_OFF:PPC, :],
            tl_vec[:].rearrange("p (t d) -> p t d", t=1),
            tl_tpl[:].rearrange("p (t k) -> p t k", t=1),
            tl_msk[:],
            tl_mcv[:],
        )

        # staggered pipeline: group g's scalar chain + gathers are emitted
        # under group g+1's loads/subs/reduces
        groups = [list(range(g, min(g + G, NCHUNK))) for g in range(0, NCHUNK, G)]
        prev = None
        for cis in groups:
            st = part_load(cis)
            if prev is not None:
                part_chain(prev)
            prev = st
        part_chain(prev)

        nc.sync.dma_start(
            o_vec[0:MAIN, :].rearrange("(p q) d -> p (q d)", p=128), st_vec[:]
        )
        # template extraction: partition p's valid gather column is i == p%16
        if sim_safe:
            # DVE one-hot extract into dense staging (CoreSim can't check the
            # partition-strided DMAs below)
            st_tpl = sp.tile([128, TPART * K2], F16, tag="st_tpl")
            imkk_t = cpool.tile([128, 16 * K2], F16, tag="imkk_t")
            nc.sync.dma_start(imkk_t[:], imaskk_d)
            for ci in range(NCHUNK):
                sl = st_tplg[:, ci * (TCH * 16 * K2) : (ci + 1) * (TCH * 16 * K2)]
                tp = wp.tile([128, TCH * 16 * K2], F16, tag="tp_ext")
                nc.vector.tensor_tensor(
                    tp[:].rearrange("p (t i k) -> p t i k", i=16, k=K2),
                    sl.rearrange("p (t i k) -> p t i k", i=16, k=K2),
                    imkk_t[:]
                    .rearrange("p (i k) -> p i k", k=K2)
                    .unsqueeze(1)
                    .broadcast_to([128, TCH, 16, K2]),
                    op=Alu.mult,
                )
                with nc.allow_low_precision("one-hot extract"):
                    nc.vector.tensor_reduce(
                        st_tpl[
                            :, ci * TCH * K2 : (ci + 1) * TCH * K2
                        ].rearrange("p (t k) -> p t k", k=K2),
                        tp[:].rearrange("p (t i k) -> p t k i", i=16, k=K2),
                        axis=AX.X,
                        op=Alu.add,
                    )
            nc.sync.dma_start(
                o_tpl[0:MAIN, :].rearrange("(p q) k -> p (q k)", p=128), st_tpl[:]
            )
        else:
            # 16 partition-strided DMAs, one per residue class j
            o_tpl6 = o_tpl[0:MAIN, :].rearrange(
                "(q j c t) k -> q j c t k", j=16, c=NCHUNK, t=TCH
            )
            for j in range(16):
                sb = st_tplg[j::16].rearrange(
                    "q (c t i k) -> q c t i k", c=NCHUNK, t=TCH, i=16
                )[:, :, :, j : j + 1, :]
                nc.sync.dma_start(o_tpl6[:, j : j + 1], sb)
        nc.sync.dma_start(
            o_msk[0:MAIN, :].rearrange("(p q) one -> p (q one)", p=128), st_msk[:]
        )
        nc.sync.dma_start(
            o_mcv[0:MAIN, :].rearrange("(p q) one -> p (q one)", p=128), st_mcv[:]
        )
        nc.sync.dma_start(o_vec[TAIL_OFF:PPC, :], tl_vec[:])
        nc.sync.dma_start(o_tpl[TAIL_OFF:PPC, :], tl_tpl[:])
        nc.sync.dma_start(o_msk[TAIL_OFF:PPC, :], tl_msk[:])
        nc.sync.dma_start(o_mcv[TAIL_OFF:PPC, :], tl_mcv[:])

    return nc


_CACHE = {}


def get_nc(sim_safe=False):
    key = ("nc", sim_safe)
    if key not in _CACHE:
        nc = bacc.Bacc("TRN2", target_bir_lowering=False, debug=False)
        build_program(nc, sim_safe=sim_safe)
        nc.compile()
        _CACHE[key] = nc
    return _CACHE[key]


def make_in_maps(w1, w2):
    w1 = np.ascontiguousarray(np.asarray(w1, dtype=np.float16).reshape(NPIX, E))
    w2 = np.ascontiguousarray(np.asarray(w2, dtype=np.float16).reshape(NPIX, E))
    rank_in = np.ascontiguousarray(np.broadcast_to(RANK64, (128, NCAND)))
    vec_in = np.ascontiguousarray(
        np.broadcast_to(VEC_DC.reshape(-1), (128, 2 * N2))
    )
    rkey_in = np.ascontiguousarray(np.broadcast_to(RKEY2, (128, NCAND)))
    vecrk_in = np.ascontiguousarray(
        np.broadcast_to(VEC_RANKC.reshape(-1), (128, NRANKC * 2))
    )
    imask_in = np.ascontiguousarray(IMASK2)
    offs_in = np.ascontiguousarray(np.broadcast_to(OFFS50, (128, TCH)))
    in_maps = []
    for c in range(NCORES):
        sl = slice(c * PPC, (c + 1) * PPC)
        in_maps.append(
            {
                "w1": np.ascontiguousarray(w1[sl]),
                "w2": np.ascontiguousarray(w2[sl]),
                "rank50": rank_in,
                "vecdc": vec_in,
                "rkey2": rkey_in,
                "vecrk": vecrk_in,
                "imask2": imask_in,
                "imaskk": np.ascontiguousarray(IMASKK),
                "offs50": offs_in,
            }
        )
    return in_maps


def assemble(results):
    vec = np.concatenate([results[c]["o_vec"] for c in range(NCORES)])
    tpl = np.concatenate([results[c]["o_tpl"] for c in range(NCORES)])
    msk = np.concatenate([results[c]["o_msk"] for c in range(NCORES)])
    mcv = np.concatenate([results[c]["o_mcv"] for c in range(NCORES)])
    return (
        vec.reshape(B, H, W, 2).astype(np.float16),
        tpl.reshape(B, H, W, 1, K2).astype(np.float16),
        msk.reshape(B, H, W, 1).astype(bool),
        mcv.reshape(B, H, W, 1).astype(np.int32),
    )


def kernel(w1, w2):
    nc = get_nc()
    in_maps = make_in_maps(w1, w2)
    res = run_bass_kernel_spmd(nc, in_maps, list(range(NCORES)))
    return assemble(res.results)


if __name__ == "__main__":
    rng = np.random.default_rng(0)
    w1 = rng.integers(0, 256, (B, H, W, NCAND, K2)).astype(np.float16)
    w2 = rng.integers(0, 256, (B, H, W, NCAND, K2)).astype(np.float16)
    outs = kernel(w1=w1, w2=w2)
    for o in outs:
        print(o.shape, o.dtype)


# revision 34
# speedup vs baseline: 1.0267x; 1.0267x over previous
"""Trainium2 Bass kernel: block-match SAD cost volume + spiral-tie-break argmin.

Problem (nn_CalculateVector): inputs w1, w2 [1,270,480,50,16] f16 (integer
values 0..255).  Per pixel: SAD cost over K2=16 for 50 candidates, argmin
over the 49 block-match candidates with center-out-spiral tie-break, then
input-MV override, LUT vector output, and template gather.

Strategy: fully data-parallel over the 129600 pixels, 16200 per NeuronCore.
Within a core, pixels are laid out partition-major (partition p owns 126
consecutive pixels) and processed in chunks of TCH pixels per partition.

The spiral argmin is computed exactly with a fused key: key[c] = cost[c] +
RANK[c]/64 (RANK = spiral rank, unique per candidate; cost is an integer
<= 4080 so the fp32 key is exact and min(key) implements first-occurrence-
in-spiral-order argmin).  The input-MV candidate gets rank 63 which makes
min/compare against it implement the reference's strict `<` semantics.

Gathers (VEC_LUT[argmin], w1[argmin]) are computed as one-hot dot products
on the vector engine (eq = key == minkey is exactly one-hot since keys are
unique within a pixel).
"""

import numpy as np

import concourse.bass as bass
import concourse.tile as tile
from concourse import bacc, mybir
from concourse.bass_utils import run_bass_kernel_spmd

SR = 3
NSIDE = 2 * SR + 1
N2 = NSIDE * NSIDE           # 49
K2 = 16
NCAND = N2 + 1               # 50
E = NCAND * K2               # 800 elements per pixel
B, H, W = 1, 270, 480
NPIX = B * H * W             # 129600
NCORES = 8
PPC = NPIX // NCORES         # 16200
TPART = PPC // 128           # 126
MAIN = TPART * 128           # 16128
TAIL_OFF = PPC - 128         # 16072 (tail tile overlaps main region; same values)
TCH = 6                      # pixels per partition per chunk
NCHUNK = TPART // TCH        # 21
G = 3                        # chunks per batched scalar-chain group

F16 = mybir.dt.float16
F32 = mybir.dt.float32
I32 = mybir.dt.int32
U8 = mybir.dt.uint8
Alu = mybir.AluOpType
AX = mybir.AxisListType


def _spiral_order(sz):
    n = 2 * sz + 1
    i = j = 0
    order = [(j + sz) * n + (i + sz)]
    dirs = [(1, 0), (0, 1), (-1, 0), (0, -1)]
    d, step = 0, 1
    while len(order) < n * n:
        for _ in range(2):
            di, dj = dirs[d]
            for _ in range(step):
                i += di
                j += dj
                if abs(i) <= sz and abs(j) <= sz:
                    order.append((j + sz) * n + (i + sz))
            d = (d + 1) % 4
        step += 1
    return np.asarray(order, dtype=np.int32)


def _vec_lut(sz):
    rng = np.arange(-sz, sz + 1)
    jj, ii = np.meshgrid(rng, rng, indexing="ij")
    return (-1.0 * np.stack([jj, ii], axis=-1).reshape(-1, 2)).astype(np.float16)


SPIRAL = _spiral_order(SR)                       # [49]
RANK = np.empty(N2, np.int64)
RANK[SPIRAL] = np.arange(N2)                     # inverse permutation
VEC = _vec_lut(SR)                               # [49, 2] f16, original order
RANK64 = np.zeros(NCAND, np.float32)
RANK64[:N2] = RANK / 64.0
RANK64[N2] = 63.0 / 64.0
VEC_DC = np.ascontiguousarray(VEC.T)             # [2, 49] (d, c) layout

# v2 encoding: key[c] = cost[c] + (RANK[c]*64 + c)/4096 (exact in fp32;
# ordering by RANK dominates so min() still tie-breaks by spiral rank, and
# frac*4096 = RANK*64 + c recovers the winning candidate).  MV candidate gets
# frac 4032/4096 = 63*64/4096 which preserves the strict-< mask semantics.
NRANKC = N2 * 64 + 64                            # 3200 table entries (max idx 3120)
RKEY2 = np.zeros(NCAND, np.float32)
RKEY2[:N2] = (RANK * 64 + np.arange(N2)) / 4096.0
RKEY2[N2] = 4032.0 / 4096.0
VEC_RANKC = np.zeros((NRANKC, 2), np.float16)    # rankc -> VEC[c]
for _c in range(N2):
    VEC_RANKC[RANK[_c] * 64 + _c] = VEC[_c]
IMASK2 = np.zeros((128, 16 * 2), np.float16)     # diag extract: i == p%16
for _p in range(128):
    IMASK2[_p, (_p % 16) * 2 : (_p % 16) * 2 + 2] = 1.0
IMASKK = np.zeros((128, 16 * K2), np.float16)    # same at K2 granularity
for _p in range(128):
    IMASKK[_p, (_p % 16) * K2 : (_p % 16 + 1) * K2] = 1.0
OFFS50 = np.tile(np.arange(TCH, dtype=np.float32) * NCAND, G)  # + t*50 per chunk


I16 = mybir.dt.int16


def build_program(nc, sim_safe=False):
    w1 = nc.dram_tensor("w1", [PPC, E], F16, kind="ExternalInput").ap()
    w2 = nc.dram_tensor("w2", [PPC, E], F16, kind="ExternalInput").ap()
    rank_d = nc.dram_tensor("rank50", [128, NCAND], F32, kind="ExternalInput").ap()
    vec_d = nc.dram_tensor("vecdc", [128, 2 * N2], F16, kind="ExternalInput").ap()
    rkey_d = nc.dram_tensor("rkey2", [128, NCAND], F32, kind="ExternalInput").ap()
    vecrk_d = nc.dram_tensor(
        "vecrk", [128, NRANKC * 2], F16, kind="ExternalInput"
    ).ap()
    imask_d = nc.dram_tensor("imask2", [128, 32], F16, kind="ExternalInput").ap()
    imaskk_d = nc.dram_tensor(
        "imaskk", [128, 16 * K2], F16, kind="ExternalInput"
    ).ap()
    offs_d = nc.dram_tensor(
        "offs50", [128, G * TCH], F32, kind="ExternalInput"
    ).ap()
    o_vec = nc.dram_tensor("o_vec", [PPC, 2], F16, kind="ExternalOutput").ap()
    o_tpl = nc.dram_tensor("o_tpl", [PPC, K2], F16, kind="ExternalOutput").ap()
    o_msk = nc.dram_tensor("o_msk", [PPC, 1], U8, kind="ExternalOutput").ap()
    o_mcv = nc.dram_tensor("o_mcv", [PPC, 1], I32, kind="ExternalOutput").ap()

    from contextlib import ExitStack

    with tile.TileContext(nc) as tc, ExitStack() as ctx:
        cpool = ctx.enter_context(tc.tile_pool(name="const", bufs=1))
        w1p = ctx.enter_context(
            tc.tile_pool(name="w1p", bufs=(4 if sim_safe else 2 * G + 1))
        )
        w2p = ctx.enter_context(tc.tile_pool(name="w2p", bufs=3))
        wp = ctx.enter_context(tc.tile_pool(name="work", bufs=2))
        sp = ctx.enter_context(tc.tile_pool(name="stage", bufs=1))

        rank_t = cpool.tile([128, NCAND], F32, tag="rank_t")
        nc.sync.dma_start(rank_t[:], rank_d)
        vec_t = cpool.tile([128, 2 * N2], F16, tag="vec_t")
        nc.sync.dma_start(vec_t[:], vec_d)
        rkey_t = cpool.tile([128, NCAND], F32, tag="rkey_t")
        nc.sync.dma_start(rkey_t[:], rkey_d)
        vecrk_t = cpool.tile([128, NRANKC * 2], F16, tag="vecrk_t")
        nc.sync.dma_start(vecrk_t[:], vecrk_d)
        imask_t = cpool.tile([128, 32], F16, tag="imask_t")
        nc.sync.dma_start(imask_t[:], imask_d)
        offs_t = cpool.tile([128, TCH], F32, tag="offs_t")
        nc.sync.dma_start(offs_t[:], offs_d)

        st_vec = sp.tile([128, TPART * 2], F16, tag="st_vec")
        st_msk = sp.tile([128, TPART], U8, tag="st_msk")
        st_mcv = sp.tile([128, TPART], I32, tag="st_mcv")
        # gathered templates, (chunk, t, i, k) per partition; valid at i==p%16
        st_tplg = sp.tile([128, NCHUNK * TCH * 16 * K2], F16, tag="st_tplg")

        w1m = w1[0:MAIN, :].rearrange("(p q) e -> p (q e)", p=128)
        w2m = w2[0:MAIN, :].rearrange("(p q) e -> p (q e)", p=128)

        def chunk(T, src1, src2, ovec, otpl, omsk, omcv):
            # v1 path (one-hot dot products on DVE) — used for the tail only.
            w1t = w1p.tile([128, T * E], F16, tag="w1t")
            nc.sync.dma_start(w1t[:], src1)
            w2t = w2p.tile([128, T * E], F16, tag="w2t")
            nc.sync.dma_start(w2t[:], src2)

            diff = wp.tile([128, T * E], F16, tag="diff_v1")
            nc.vector.tensor_tensor(diff[:], w1t[:], w2t[:], op=Alu.subtract)

            cost = wp.tile([128, T * NCAND], F32, tag="cost_v1")
            cost3 = cost[:].rearrange("p (t c) -> p t c", c=NCAND)
            nc.vector.tensor_reduce(
                cost3,
                diff[:].rearrange("p (t c k) -> p t c k", c=NCAND, k=K2),
                axis=AX.X,
                op=Alu.add,
                apply_absolute_value=True,
            )

            key = wp.tile([128, T * NCAND], F32, tag="key_v1")
            key3 = key[:].rearrange("p (t c) -> p t c", c=NCAND)
            nc.vector.tensor_tensor(
                key3,
                cost3,
                rank_t[:].unsqueeze(1).broadcast_to([128, T, NCAND]),
                op=Alu.add,
            )

            minkbm = wp.tile([128, T], F32, tag="minkbm_v1")
            nc.vector.tensor_reduce(
                minkbm[:], key3[:, :, 0:N2], axis=AX.X, op=Alu.min
            )

            kmv = key3[:, :, N2:NCAND].rearrange("p t one -> p (t one)")
            mk50 = wp.tile([128, T], F32, tag="mk50_v1")
            nc.vector.tensor_tensor(mk50[:], kmv, minkbm[:], op=Alu.min)

            maskf = wp.tile([128, T], F32, tag="maskf_v1")
            nc.vector.tensor_tensor(maskf[:], kmv, minkbm[:], op=Alu.is_lt)
            nc.scalar.copy(omsk, maskf[:])

            nc.vector.tensor_reduce(omcv, cost3, axis=AX.X, op=Alu.min)

            keyct = key[:].rearrange("p (t c) -> p c t", c=NCAND)
            eq50 = wp.tile([128, NCAND * T], F16, tag="eq50_v1")
            eq50v = eq50[:].rearrange("p (c t) -> p c t", t=T)
            nc.vector.tensor_tensor(
                eq50v,
                keyct,
                mk50[:].unsqueeze(1).broadcast_to([128, NCAND, T]),
                op=Alu.is_equal,
            )
            eq49 = wp.tile([128, N2 * T], F16, tag="eq49_v1")
            eq49v = eq49[:].rearrange("p (c t) -> p c t", t=T)
            nc.vector.tensor_tensor(
                eq49v,
                keyct[:, 0:N2, :],
                minkbm[:].unsqueeze(1).broadcast_to([128, N2, T]),
                op=Alu.is_equal,
            )

            prodv = wp.tile([128, T * 2 * N2], F16, tag="prodv_v1")
            prodv4 = prodv[:].rearrange("p (t d c) -> p t d c", d=2, c=N2)
            e49_tdc = (
                eq49[:]
                .rearrange("p (c t) -> p t c", t=T)
                .unsqueeze(2)
                .broadcast_to([128, T, 2, N2])
            )
            vdc = (
                vec_t[:]
                .rearrange("p (d c) -> p d c", c=N2)
                .unsqueeze(1)
                .broadcast_to([128, T, 2, N2])
            )
            nc.vector.tensor_tensor(prodv4, e49_tdc, vdc, op=Alu.mult)
            with nc.allow_low_precision("one-hot dot, sums are exact"):
                nc.vector.tensor_reduce(ovec, prodv4, axis=AX.X, op=Alu.add)

            prod8 = wp.tile([128, T * K2 * NCAND], F16, tag="prod8_v1")
            prod84 = prod8[:].rearrange("p (t k c) -> p t k c", k=K2, c=NCAND)
            w1_tkc = w1t[:].rearrange("p (t c k) -> p t k c", c=NCAND, k=K2)
            eq50_tkc = (
                eq50[:]
                .rearrange("p (c t) -> p t c", t=T)
                .unsqueeze(2)
                .broadcast_to([128, T, K2, NCAND])
            )
            nc.vector.tensor_tensor(prod84, w1_tkc, eq50_tkc, op=Alu.mult)
            with nc.allow_low_precision("one-hot dot, sums are exact"):
                nc.vector.tensor_reduce(otpl, prod84, axis=AX.X, op=Alu.add)

        # ---- v3 main path: per-chunk streaming (sub+reduce) with the scalar
        # chain batched per group of G chunks, GPSIMD doing gathers and the
        # small tensor-tensor chain, and partition-strided extraction DMAs.
        def part_load(cis):
            # loads + subtract + grouped SAD reduce for a group of chunks
            w1ts = []
            cost_w = wp.tile([128, G * TCH * NCAND], F32, tag="cost_w")
            for l, ci in enumerate(cis):
                t0 = ci * TCH
                sl = slice(t0 * E, (t0 + TCH) * E)
                w1t = w1p.tile([128, TCH * E], F16, tag="w1t")
                nc.sync.dma_start(w1t[:], w1m[:, sl])
                w2t = w2p.tile([128, TCH * E], F16, tag="w2t")
                nc.sync.dma_start(w2t[:], w2m[:, sl])
                w1ts.append(w1t)
                # diff in place of w2t (w2 is dead after this)
                nc.vector.tensor_tensor(w2t[:], w1t[:], w2t[:], op=Alu.subtract)
                nc.vector.tensor_reduce(
                    cost_w[:, l * TCH * NCAND : (l + 1) * TCH * NCAND].rearrange(
                        "p (t c) -> p t c", c=NCAND
                    ),
                    w2t[:].rearrange("p (t c k) -> p t c k", c=NCAND, k=K2),
                    axis=AX.X,
                    op=Alu.add,
                    apply_absolute_value=True,
                )
            return {"cis": cis, "w1ts": w1ts, "cost_w": cost_w}

        def part_chain(st):
            cis, w1ts, cost_w = st["cis"], st["w1ts"], st["cost_w"]
            W = len(cis) * TCH
            cost3 = cost_w[:, : W * NCAND].rearrange("p (t c) -> p t c", c=NCAND)
            key_w = wp.tile([128, G * TCH * NCAND], F32, tag="key_w")
            key3 = key_w[:, : W * NCAND].rearrange("p (t c) -> p t c", c=NCAND)
            nc.vector.tensor_tensor(
                key3,
                cost3,
                rkey_t[:].unsqueeze(1).broadcast_to([128, W, NCAND]),
                op=Alu.add,
            )
            minkbm = wp.tile([128, G * TCH], F32, tag="minkbm")
            nc.vector.tensor_reduce(
                minkbm[:, :W], key3[:, :, 0:N2], axis=AX.X, op=Alu.min
            )
            mincbm = wp.tile([128, G * TCH], F32, tag="mincbm")
            nc.vector.tensor_reduce(
                mincbm[:, :W], cost3[:, :, 0:N2], axis=AX.X, op=Alu.min
            )
            kmv = key3[:, :, N2:NCAND].rearrange("p t one -> p (t one)")
            cmv = cost3[:, :, N2:NCAND].rearrange("p t one -> p (t one)")
            maskf = wp.tile([128, G * TCH], F32, tag="maskf")
            nc.vector.tensor_tensor(maskf[:, :W], kmv, minkbm[:, :W], op=Alu.is_lt)
            p0 = cis[0] * TCH
            nc.scalar.copy(st_msk[:, p0 : p0 + W], maskf[:, :W])
            nc.vector.tensor_tensor(
                st_mcv[:, p0 : p0 + W], cmv, mincbm[:, :W], op=Alu.min
            )
            frac = wp.tile([128, G * TCH], F32, tag="frac")
            nc.gpsimd.tensor_tensor(
                frac[:, :W], minkbm[:, :W], mincbm[:, :W], op=Alu.subtract
            )
            rci = wp.tile([128, G * TCH], I16, tag="rci")
            nc.vector.tensor_scalar(
                rci[:, :W], frac[:, :W], 4096.0, None, op0=Alu.mult
            )
            gv = wp.tile([128, G * TCH * 32], F16, tag="gv")
            nc.gpsimd.ap_gather(
                gv[:, : W * 32],
                vecrk_t[:],
                rci[:, :W],
                channels=128,
                num_elems=NRANKC,
                d=2,
                num_idxs=16 * W,
            )
            vprod = wp.tile([128, G * TCH * 32], F16, tag="vprod")
            nc.gpsimd.tensor_tensor(
                vprod[:, : W * 32].rearrange("p (t i d) -> p t i d", i=16, d=2),
                gv[:, : W * 32].rearrange("p (t i d) -> p t i d", i=16, d=2),
                imask_t[:]
                .rearrange("p (i d) -> p i d", d=2)
                .unsqueeze(1)
                .broadcast_to([128, W, 16, 2]),
                op=Alu.mult,
            )
            ovec = st_vec[:, p0 * 2 : (p0 + W) * 2].rearrange(
                "p (t d) -> p t d", d=2
            )
            with nc.allow_low_precision("one-hot extract, sums exact"):
                nc.vector.tensor_reduce(
                    ovec,
                    vprod[:, : W * 32].rearrange("p (t i d) -> p t d i", i=16, d=2),
                    axis=AX.X,
                    op=Alu.add,
                )
            vsl = st_vec[:, p0 * 2 : (p0 + W) * 2].rearrange("p (t d) -> p t d", d=2)
            v0 = vsl[:, :, 0:1].rearrange("p t one -> p (t one)")
            v1 = vsl[:, :, 1:2].rearrange("p t one -> p (t one)")
            # s = 7*v0 + v1; c = 24 - s; 49 - c = 25 + s
            s_w = wp.tile([128, G * TCH], F32, tag="s_w")
            nc.vector.scalar_tensor_tensor(
                s_w[:, :W], v0, 7.0, v1, op0=Alu.mult, op1=Alu.add
            )
            cidx = wp.tile([128, G * TCH], F32, tag="cidx")
            nc.vector.tensor_scalar(
                cidx[:, :W], s_w[:, :W], -1.0, 24.0, op0=Alu.mult, op1=Alu.add
            )
            t1 = wp.tile([128, G * TCH], F32, tag="t1")
            nc.vector.tensor_scalar(
                t1[:, :W], s_w[:, :W], 1.0, 25.0, op0=Alu.mult, op1=Alu.add
            )
            nc.gpsimd.tensor_tensor(t1[:, :W], t1[:, :W], maskf[:, :W], op=Alu.mult)
            nc.gpsimd.tensor_tensor(t1[:, :W], t1[:, :W], cidx[:, :W], op=Alu.add)
            for l, ci in enumerate(cis):
                # per-chunk tidx tile: ap_gather needs an aligned index base
                tidx = wp.tile([128, TCH], I16, tag=f"tidx{l}")
                nc.vector.tensor_tensor(
                    tidx[:],
                    t1[:, l * TCH : (l + 1) * TCH],
                    offs_t[:, 0:TCH],
                    op=Alu.add,
                )
                nc.gpsimd.ap_gather(
                    st_tplg[:, ci * (TCH * 16 * K2) : (ci + 1) * (TCH * 16 * K2)],
                    w1ts[l][:],
                    tidx[:],
                    channels=128,
                    num_elems=TCH * NCAND,
                    d=K2,
                    num_idxs=16 * TCH,
                )
            if not sim_safe:
                tpl_extract_dmas(cis[0], cis[-1] + 1)
            if cis[-1] + 1 == 12:
                flush_outputs(0, 12 * TCH)

        def flush_outputs(q0, q1):
            # staged vec/msk/mcv for per-partition pixel range [q0, q1)
            nc.scalar.dma_start(
                o_vec[0:MAIN, :].rearrange("(p q) d -> p q d", p=128)[:, q0:q1],
                st_vec[:, q0 * 2 : q1 * 2],
            )
            nc.scalar.dma_start(
                o_msk[0:MAIN, :].rearrange("(p q) one -> p q one", p=128)[:, q0:q1],
                st_msk[:, q0:q1],
            )
            nc.scalar.dma_start(
                o_mcv[0:MAIN, :].rearrange("(p q) one -> p q one", p=128)[:, q0:q1],
                st_mcv[:, q0:q1],
            )

        def tpl_extract_dmas(c0, c1):
            # partition-strided extraction for chunks [c0, c1)
            o6 = o_tpl[0:MAIN, :].rearrange(
                "(q j c t) k -> q j c t k", j=16, c=NCHUNK, t=TCH
            )
            for j in range(16):
                sb = st_tplg[j::16].rearrange(
                    "q (c t i k) -> q c t i k", c=NCHUNK, t=TCH, i=16
                )[:, c0:c1, :, j : j + 1, :]
                nc.scalar.dma_start(o6[:, j : j + 1, c0:c1], sb)

        # tail first so it overlaps the main stream: 128 pixels
        # [TAIL_OFF, PPC), one per partition
        tl_vec = sp.tile([128, 2], F16, tag="tl_vec")
        tl_tpl = sp.tile([128, K2], F16, tag="tl_tpl")
        tl_msk = sp.tile([128, 1], U8, tag="tl_msk")
        tl_mcv = sp.tile([128, 1], I32, tag="tl_mcv")
        chunk(
            1,
            w1[TAIL_OFF:PPC, :],
            w2[TAIL_OFF:PPC, :],
            tl_vec[:].rearrange("p (t d) -> p t d", t=1),
            tl_tpl[:].rearrange("p (t k) -> p t k", t=1),
            tl_msk[:],
            tl_mcv[:],
        )

        # staggered pipeline: group g's scalar chain + gathers are emitted
        # under group g+1's loads/subs/reduces
        groups = []
        _g = 0
        while _g < NCHUNK:
            n = G if NCHUNK - _g > 3 else (2 if NCHUNK - _g == 3 else NCHUNK - _g)
            groups.append(list(range(_g, _g + n)))
            _g += n
        prev = None
        for cis in groups:
            st = part_load(cis)
            if prev is not None:
                part_chain(prev)
            prev = st
        part_chain(prev)

        flush_outputs(12 * TCH, TPART)
        # template extraction: partition p's valid gather column is i == p%16
        if sim_safe:
            # DVE one-hot extract into dense staging (CoreSim can't check the
            # partition-strided DMAs below)
            st_tpl = sp.tile([128, TPART * K2], F16, tag="st_tpl")
            imkk_t = cpool.tile([128, 16 * K2], F16, tag="imkk_t")
            nc.sync.dma_start(imkk_t[:], imaskk_d)
            for ci in range(NCHUNK):
                sl = st_tplg[:, ci * (TCH * 16 * K2) : (ci + 1) * (TCH * 16 * K2)]
                tp = wp.tile([128, TCH * 16 * K2], F16, tag="tp_ext")
                nc.vector.tensor_tensor(
                    tp[:].rearrange("p (t i k) -> p t i k", i=16, k=K2),
                    sl.rearrange("p (t i k) -> p t i k", i=16, k=K2),
                    imkk_t[:]
                    .rearrange("p (i k) -> p i k", k=K2)
                    .unsqueeze(1)
                    .broadcast_to([128, TCH, 16, K2]),
                    op=Alu.mult,
                )
                with nc.allow_low_precision("one-hot extract"):
                    nc.vector.tensor_reduce(
                        st_tpl[
                            :, ci * TCH * K2 : (ci + 1) * TCH * K2
                        ].rearrange("p (t k) -> p t k", k=K2),
                        tp[:].rearrange("p (t i k) -> p t k i", i=16, k=K2),
                        axis=AX.X,
                        op=Alu.add,
                    )
            nc.sync.dma_start(
                o_tpl[0:MAIN, :].rearrange("(p q) k -> p (q k)", p=128), st_tpl[:]
            )
        nc.sync.dma_start(o_vec[TAIL_OFF:PPC, :], tl_vec[:])
        nc.sync.dma_start(o_tpl[TAIL_OFF:PPC, :], tl_tpl[:])
        nc.sync.dma_start(o_msk[TAIL_OFF:PPC, :], tl_msk[:])
        nc.sync.dma_start(o_mcv[TAIL_OFF:PPC, :], tl_mcv[:])

    return nc


_CACHE = {}


def get_nc(sim_safe=False):
    key = ("nc", sim_safe)
    if key not in _CACHE:
        nc = bacc.Bacc("TRN2", target_bir_lowering=False, debug=False)
        build_program(nc, sim_safe=sim_safe)
        nc.compile()
        _CACHE[key] = nc
    return _CACHE[key]


def make_in_maps(w1, w2):
    w1 = np.ascontiguousarray(np.asarray(w1, dtype=np.float16).reshape(NPIX, E))
    w2 = np.ascontiguousarray(np.asarray(w2, dtype=np.float16).reshape(NPIX, E))
    rank_in = np.ascontiguousarray(np.broadcast_to(RANK64, (128, NCAND)))
    vec_in = np.ascontiguousarray(
        np.broadcast_to(VEC_DC.reshape(-1), (128, 2 * N2))
    )
    rkey_in = np.ascontiguousarray(np.broadcast_to(RKEY2, (128, NCAND)))
    vecrk_in = np.ascontiguousarray(
        np.broadcast_to(VEC_RANKC.reshape(-1), (128, NRANKC * 2))
    )
    imask_in = np.ascontiguousarray(IMASK2)
    offs_in = np.ascontiguousarray(np.broadcast_to(OFFS50, (128, TCH)))
    in_maps = []
    for c in range(NCORES):
        sl = slice(c * PPC, (c + 1) * PPC)
        in_maps.append(
            {
                "w1": np.ascontiguousarray(w1[sl]),
                "w2": np.ascontiguousarray(w2[sl]),
                "rank50": rank_in,
                "vecdc": vec_in,
                "rkey2": rkey_in,
                "vecrk": vecrk_in,
                "imask2": imask_in,
                "imaskk": np.ascontiguousarray(IMASKK),
                "offs50": offs_in,
            }
        )
    return in_maps


def assemble(results):
    vec = np.concatenate([results[c]["o_vec"] for c in range(NCORES)])
    tpl = np.concatenate([results[c]["o_tpl"] for c in range(NCORES)])
    msk = np.concatenate([results[c]["o_msk"] for c in range(NCORES)])
    mcv = np.concatenate([results[c]["o_mcv"] for c in range(NCORES)])
    return (
        vec.reshape(B, H, W, 2).astype(np.float16),
        tpl.reshape(B, H, W, 1, K2).astype(np.float16),
        msk.reshape(B, H, W, 1).astype(bool),
        mcv.reshape(B, H, W, 1).astype(np.int32),
    )


def kernel(w1, w2):
    nc = get_nc()
    in_maps = make_in_maps(w1, w2)
    res = run_bass_kernel_spmd(nc, in_maps, list(range(NCORES)))
    return assemble(res.results)


if __name__ == "__main__":
    rng = np.random.default_rng(0)
    w1 = rng.integers(0, 256, (B, H, W, NCAND, K2)).astype(np.float16)
    w2 = rng.integers(0, 256, (B, H, W, NCAND, K2)).astype(np.float16)
    outs = kernel(w1=w1, w2=w2)
    for o in outs:
        print(o.shape, o.dtype)


# revision 35
# speedup vs baseline: 1.0356x; 1.0087x over previous
"""Trainium2 Bass kernel: block-match SAD cost volume + spiral-tie-break argmin.

Problem (nn_CalculateVector): inputs w1, w2 [1,270,480,50,16] f16 (integer
values 0..255).  Per pixel: SAD cost over K2=16 for 50 candidates, argmin
over the 49 block-match candidates with center-out-spiral tie-break, then
input-MV override, LUT vector output, and template gather.

Strategy: fully data-parallel over the 129600 pixels, 16200 per NeuronCore.
Within a core, pixels are laid out partition-major (partition p owns 126
consecutive pixels) and processed in chunks of TCH pixels per partition.

The spiral argmin is computed exactly with a fused key: key[c] = cost[c] +
RANK[c]/64 (RANK = spiral rank, unique per candidate; cost is an integer
<= 4080 so the fp32 key is exact and min(key) implements first-occurrence-
in-spiral-order argmin).  The input-MV candidate gets rank 63 which makes
min/compare against it implement the reference's strict `<` semantics.

Gathers (VEC_LUT[argmin], w1[argmin]) are computed as one-hot dot products
on the vector engine (eq = key == minkey is exactly one-hot since keys are
unique within a pixel).
"""

import numpy as np

import concourse.bass as bass
import concourse.tile as tile
from concourse import bacc, mybir
from concourse.bass_utils import run_bass_kernel_spmd

SR = 3
NSIDE = 2 * SR + 1
N2 = NSIDE * NSIDE           # 49
K2 = 16
NCAND = N2 + 1               # 50
E = NCAND * K2               # 800 elements per pixel
B, H, W = 1, 270, 480
NPIX = B * H * W             # 129600
NCORES = 8
PPC = NPIX // NCORES         # 16200
TPART = PPC // 128           # 126
MAIN = TPART * 128           # 16128
TAIL_OFF = PPC - 128         # 16072 (tail tile overlaps main region; same values)
TCH = 6                      # pixels per partition per chunk
NCHUNK = TPART // TCH        # 21
G = 3                        # chunks per batched scalar-chain group

F16 = mybir.dt.float16
F32 = mybir.dt.float32
I32 = mybir.dt.int32
U8 = mybir.dt.uint8
Alu = mybir.AluOpType
AX = mybir.AxisListType


def _spiral_order(sz):
    n = 2 * sz + 1
    i = j = 0
    order = [(j + sz) * n + (i + sz)]
    dirs = [(1, 0), (0, 1), (-1, 0), (0, -1)]
    d, step = 0, 1
    while len(order) < n * n:
        for _ in range(2):
            di, dj = dirs[d]
            for _ in range(step):
                i += di
                j += dj
                if abs(i) <= sz and abs(j) <= sz:
                    order.append((j + sz) * n + (i + sz))
            d = (d + 1) % 4
        step += 1
    return np.asarray(order, dtype=np.int32)


def _vec_lut(sz):
    rng = np.arange(-sz, sz + 1)
    jj, ii = np.meshgrid(rng, rng, indexing="ij")
    return (-1.0 * np.stack([jj, ii], axis=-1).reshape(-1, 2)).astype(np.float16)


SPIRAL = _spiral_order(SR)                       # [49]
RANK = np.empty(N2, np.int64)
RANK[SPIRAL] = np.arange(N2)                     # inverse permutation
VEC = _vec_lut(SR)                               # [49, 2] f16, original order
RANK64 = np.zeros(NCAND, np.float32)
RANK64[:N2] = RANK / 64.0
RANK64[N2] = 63.0 / 64.0
VEC_DC = np.ascontiguousarray(VEC.T)             # [2, 49] (d, c) layout

# v2 encoding: key[c] = cost[c] + (RANK[c]*64 + c)/4096 (exact in fp32;
# ordering by RANK dominates so min() still tie-breaks by spiral rank, and
# frac*4096 = RANK*64 + c recovers the winning candidate).  MV candidate gets
# frac 4032/4096 = 63*64/4096 which preserves the strict-< mask semantics.
NRANKC = N2 * 64 + 64                            # 3200 table entries (max idx 3120)
RKEY2 = np.zeros(NCAND, np.float32)
RKEY2[:N2] = (RANK * 64 + np.arange(N2)) / 4096.0
RKEY2[N2] = 4032.0 / 4096.0
VEC_RANKC = np.zeros((NRANKC, 2), np.float16)    # rankc -> VEC[c]
for _c in range(N2):
    VEC_RANKC[RANK[_c] * 64 + _c] = VEC[_c]
IMASK2 = np.zeros((128, 16 * 2), np.float16)     # diag extract: i == p%16
for _p in range(128):
    IMASK2[_p, (_p % 16) * 2 : (_p % 16) * 2 + 2] = 1.0
IMASKK = np.zeros((128, 16 * K2), np.float16)    # same at K2 granularity
for _p in range(128):
    IMASKK[_p, (_p % 16) * K2 : (_p % 16 + 1) * K2] = 1.0
OFFS50 = np.tile(np.arange(TCH, dtype=np.float32) * NCAND, G)  # + t*50 per chunk


I16 = mybir.dt.int16


def build_program(nc, sim_safe=False):
    w1 = nc.dram_tensor("w1", [PPC, E], F16, kind="ExternalInput").ap()
    w2 = nc.dram_tensor("w2", [PPC, E], F16, kind="ExternalInput").ap()
    rank_d = nc.dram_tensor("rank50", [128, NCAND], F32, kind="ExternalInput").ap()
    vec_d = nc.dram_tensor("vecdc", [128, 2 * N2], F16, kind="ExternalInput").ap()
    rkey_d = nc.dram_tensor("rkey2", [128, NCAND], F32, kind="ExternalInput").ap()
    vecrk_d = nc.dram_tensor(
        "vecrk", [128, NRANKC * 2], F16, kind="ExternalInput"
    ).ap()
    imask_d = nc.dram_tensor("imask2", [128, 32], F16, kind="ExternalInput").ap()
    imaskk_d = nc.dram_tensor(
        "imaskk", [128, 16 * K2], F16, kind="ExternalInput"
    ).ap()
    offs_d = nc.dram_tensor(
        "offs50", [128, G * TCH], F32, kind="ExternalInput"
    ).ap()
    o_vec = nc.dram_tensor("o_vec", [PPC, 2], F16, kind="ExternalOutput").ap()
    o_tpl = nc.dram_tensor("o_tpl", [PPC, K2], F16, kind="ExternalOutput").ap()
    o_msk = nc.dram_tensor("o_msk", [PPC, 1], U8, kind="ExternalOutput").ap()
    o_mcv = nc.dram_tensor("o_mcv", [PPC, 1], I32, kind="ExternalOutput").ap()

    from contextlib import ExitStack

    with tile.TileContext(nc) as tc, ExitStack() as ctx:
        cpool = ctx.enter_context(tc.tile_pool(name="const", bufs=1))
        w1p = ctx.enter_context(
            tc.tile_pool(name="w1p", bufs=(4 if sim_safe else 2 * G + 1))
        )
        w2p = ctx.enter_context(tc.tile_pool(name="w2p", bufs=3))
        wp = ctx.enter_context(tc.tile_pool(name="work", bufs=2))
        sp = ctx.enter_context(tc.tile_pool(name="stage", bufs=1))

        rank_t = cpool.tile([128, NCAND], F32, tag="rank_t")
        nc.sync.dma_start(rank_t[:], rank_d)
        vec_t = cpool.tile([128, 2 * N2], F16, tag="vec_t")
        nc.sync.dma_start(vec_t[:], vec_d)
        rkey_t = cpool.tile([128, NCAND], F32, tag="rkey_t")
        nc.sync.dma_start(rkey_t[:], rkey_d)
        vecrk_t = cpool.tile([128, NRANKC * 2], F16, tag="vecrk_t")
        nc.sync.dma_start(vecrk_t[:], vecrk_d)
        imask_t = cpool.tile([128, 32], F16, tag="imask_t")
        nc.sync.dma_start(imask_t[:], imask_d)
        offs_t = cpool.tile([128, TCH], F32, tag="offs_t")
        nc.sync.dma_start(offs_t[:], offs_d)

        st_vec = sp.tile([128, TPART * 2], F16, tag="st_vec")
        st_msk = sp.tile([128, TPART], U8, tag="st_msk")
        st_mcv = sp.tile([128, TPART], I32, tag="st_mcv")
        # gathered templates, (chunk, t, i, k) per partition; valid at i==p%16
        st_tplg = sp.tile([128, NCHUNK * TCH * 16 * K2], F16, tag="st_tplg")

        w1m = w1[0:MAIN, :].rearrange("(p q) e -> p (q e)", p=128)
        w2m = w2[0:MAIN, :].rearrange("(p q) e -> p (q e)", p=128)

        def chunk(T, src1, src2, ovec, otpl, omsk, omcv):
            # v1 path (one-hot dot products on DVE) — used for the tail only.
            w1t = w1p.tile([128, T * E], F16, tag="w1t")
            nc.sync.dma_start(w1t[:], src1)
            w2t = w2p.tile([128, T * E], F16, tag="w2t")
            nc.scalar.dma_start(w2t[:], src2)

            diff = wp.tile([128, T * E], F16, tag="diff_v1")
            nc.vector.tensor_tensor(diff[:], w1t[:], w2t[:], op=Alu.subtract)

            cost = wp.tile([128, T * NCAND], F32, tag="cost_v1")
            cost3 = cost[:].rearrange("p (t c) -> p t c", c=NCAND)
            nc.vector.tensor_reduce(
                cost3,
                diff[:].rearrange("p (t c k) -> p t c k", c=NCAND, k=K2),
                axis=AX.X,
                op=Alu.add,
                apply_absolute_value=True,
            )

            key = wp.tile([128, T * NCAND], F32, tag="key_v1")
            key3 = key[:].rearrange("p (t c) -> p t c", c=NCAND)
            nc.vector.tensor_tensor(
                key3,
                cost3,
                rank_t[:].unsqueeze(1).broadcast_to([128, T, NCAND]),
                op=Alu.add,
            )

            minkbm = wp.tile([128, T], F32, tag="minkbm_v1")
            nc.vector.tensor_reduce(
                minkbm[:], key3[:, :, 0:N2], axis=AX.X, op=Alu.min
            )

            kmv = key3[:, :, N2:NCAND].rearrange("p t one -> p (t one)")
            mk50 = wp.tile([128, T], F32, tag="mk50_v1")
            nc.vector.tensor_tensor(mk50[:], kmv, minkbm[:], op=Alu.min)

            maskf = wp.tile([128, T], F32, tag="maskf_v1")
            nc.vector.tensor_tensor(maskf[:], kmv, minkbm[:], op=Alu.is_lt)
            nc.scalar.copy(omsk, maskf[:])

            nc.vector.tensor_reduce(omcv, cost3, axis=AX.X, op=Alu.min)

            keyct = key[:].rearrange("p (t c) -> p c t", c=NCAND)
            eq50 = wp.tile([128, NCAND * T], F16, tag="eq50_v1")
            eq50v = eq50[:].rearrange("p (c t) -> p c t", t=T)
            nc.vector.tensor_tensor(
                eq50v,
                keyct,
                mk50[:].unsqueeze(1).broadcast_to([128, NCAND, T]),
                op=Alu.is_equal,
            )
            eq49 = wp.tile([128, N2 * T], F16, tag="eq49_v1")
            eq49v = eq49[:].rearrange("p (c t) -> p c t", t=T)
            nc.vector.tensor_tensor(
                eq49v,
                keyct[:, 0:N2, :],
                minkbm[:].unsqueeze(1).broadcast_to([128, N2, T]),
                op=Alu.is_equal,
            )

            prodv = wp.tile([128, T * 2 * N2], F16, tag="prodv_v1")
            prodv4 = prodv[:].rearrange("p (t d c) -> p t d c", d=2, c=N2)
            e49_tdc = (
                eq49[:]
                .rearrange("p (c t) -> p t c", t=T)
                .unsqueeze(2)
                .broadcast_to([128, T, 2, N2])
            )
            vdc = (
                vec_t[:]
                .rearrange("p (d c) -> p d c", c=N2)
                .unsqueeze(1)
                .broadcast_to([128, T, 2, N2])
            )
            nc.vector.tensor_tensor(prodv4, e49_tdc, vdc, op=Alu.mult)
            with nc.allow_low_precision("one-hot dot, sums are exact"):
                nc.vector.tensor_reduce(ovec, prodv4, axis=AX.X, op=Alu.add)

            prod8 = wp.tile([128, T * K2 * NCAND], F16, tag="prod8_v1")
            prod84 = prod8[:].rearrange("p (t k c) -> p t k c", k=K2, c=NCAND)
            w1_tkc = w1t[:].rearrange("p (t c k) -> p t k c", c=NCAND, k=K2)
            eq50_tkc = (
                eq50[:]
                .rearrange("p (c t) -> p t c", t=T)
                .unsqueeze(2)
                .broadcast_to([128, T, K2, NCAND])
            )
            nc.vector.tensor_tensor(prod84, w1_tkc, eq50_tkc, op=Alu.mult)
            with nc.allow_low_precision("one-hot dot, sums are exact"):
                nc.vector.tensor_reduce(otpl, prod84, axis=AX.X, op=Alu.add)

        # ---- v3 main path: per-chunk streaming (sub+reduce) with the scalar
        # chain batched per group of G chunks, GPSIMD doing gathers and the
        # small tensor-tensor chain, and partition-strided extraction DMAs.
        def part_load(cis):
            # loads + subtract + grouped SAD reduce for a group of chunks
            w1ts = []
            cost_w = wp.tile([128, G * TCH * NCAND], F32, tag="cost_w")
            for l, ci in enumerate(cis):
                t0 = ci * TCH
                sl = slice(t0 * E, (t0 + TCH) * E)
                if ci % 4 == 3:
                    eng1 = eng2 = nc.gpsimd
                elif ci % 2 == 0:
                    eng1, eng2 = nc.sync, nc.scalar
                else:
                    eng1, eng2 = nc.scalar, nc.sync
                w1t = w1p.tile([128, TCH * E], F16, tag="w1t")
                eng1.dma_start(w1t[:], w1m[:, sl])
                w2t = w2p.tile([128, TCH * E], F16, tag="w2t")
                eng2.dma_start(w2t[:], w2m[:, sl])
                w1ts.append(w1t)
                # diff in place of w2t (w2 is dead after this)
                nc.vector.tensor_tensor(w2t[:], w1t[:], w2t[:], op=Alu.subtract)
                nc.vector.tensor_reduce(
                    cost_w[:, l * TCH * NCAND : (l + 1) * TCH * NCAND].rearrange(
                        "p (t c) -> p t c", c=NCAND
                    ),
                    w2t[:].rearrange("p (t c k) -> p t c k", c=NCAND, k=K2),
                    axis=AX.X,
                    op=Alu.add,
                    apply_absolute_value=True,
                )
            return {"cis": cis, "w1ts": w1ts, "cost_w": cost_w}

        def part_chain(st):
            cis, w1ts, cost_w = st["cis"], st["w1ts"], st["cost_w"]
            W = len(cis) * TCH
            cost3 = cost_w[:, : W * NCAND].rearrange("p (t c) -> p t c", c=NCAND)
            key_w = wp.tile([128, G * TCH * NCAND], F32, tag="key_w")
            key3 = key_w[:, : W * NCAND].rearrange("p (t c) -> p t c", c=NCAND)
            nc.vector.tensor_tensor(
                key3,
                cost3,
                rkey_t[:].unsqueeze(1).broadcast_to([128, W, NCAND]),
                op=Alu.add,
            )
            minkbm = wp.tile([128, G * TCH], F32, tag="minkbm")
            nc.vector.tensor_reduce(
                minkbm[:, :W], key3[:, :, 0:N2], axis=AX.X, op=Alu.min
            )
            mincbm = wp.tile([128, G * TCH], F32, tag="mincbm")
            nc.vector.tensor_reduce(
                mincbm[:, :W], cost3[:, :, 0:N2], axis=AX.X, op=Alu.min
            )
            kmv = key3[:, :, N2:NCAND].rearrange("p t one -> p (t one)")
            cmv = cost3[:, :, N2:NCAND].rearrange("p t one -> p (t one)")
            maskf = wp.tile([128, G * TCH], F32, tag="maskf")
            nc.vector.tensor_tensor(maskf[:, :W], kmv, minkbm[:, :W], op=Alu.is_lt)
            p0 = cis[0] * TCH
            nc.scalar.copy(st_msk[:, p0 : p0 + W], maskf[:, :W])
            nc.vector.tensor_tensor(
                st_mcv[:, p0 : p0 + W], cmv, mincbm[:, :W], op=Alu.min
            )
            frac = wp.tile([128, G * TCH], F32, tag="frac")
            nc.gpsimd.tensor_tensor(
                frac[:, :W], minkbm[:, :W], mincbm[:, :W], op=Alu.subtract
            )
            rci = wp.tile([128, G * TCH], I16, tag="rci")
            nc.vector.tensor_scalar(
                rci[:, :W], frac[:, :W], 4096.0, None, op0=Alu.mult
            )
            gv = wp.tile([128, G * TCH * 32], F16, tag="gv")
            nc.gpsimd.ap_gather(
                gv[:, : W * 32],
                vecrk_t[:],
                rci[:, :W],
                channels=128,
                num_elems=NRANKC,
                d=2,
                num_idxs=16 * W,
            )
            vprod = wp.tile([128, G * TCH * 32], F16, tag="vprod")
            nc.gpsimd.tensor_tensor(
                vprod[:, : W * 32].rearrange("p (t i d) -> p t i d", i=16, d=2),
                gv[:, : W * 32].rearrange("p (t i d) -> p t i d", i=16, d=2),
                imask_t[:]
                .rearrange("p (i d) -> p i d", d=2)
                .unsqueeze(1)
                .broadcast_to([128, W, 16, 2]),
                op=Alu.mult,
            )
            ovec = st_vec[:, p0 * 2 : (p0 + W) * 2].rearrange(
                "p (t d) -> p t d", d=2
            )
            with nc.allow_low_precision("one-hot extract, sums exact"):
                nc.vector.tensor_reduce(
                    ovec,
                    vprod[:, : W * 32].rearrange("p (t i d) -> p t d i", i=16, d=2),
                    axis=AX.X,
                    op=Alu.add,
                )
            vsl = st_vec[:, p0 * 2 : (p0 + W) * 2].rearrange("p (t d) -> p t d", d=2)
            v0 = vsl[:, :, 0:1].rearrange("p t one -> p (t one)")
            v1 = vsl[:, :, 1:2].rearrange("p t one -> p (t one)")
            # s = 7*v0 + v1; c = 24 - s; 49 - c = 25 + s
            s_w = wp.tile([128, G * TCH], F32, tag="s_w")
            nc.vector.scalar_tensor_tensor(
                s_w[:, :W], v0, 7.0, v1, op0=Alu.mult, op1=Alu.add
            )
            cidx = wp.tile([128, G * TCH], F32, tag="cidx")
            nc.vector.tensor_scalar(
                cidx[:, :W], s_w[:, :W], -1.0, 24.0, op0=Alu.mult, op1=Alu.add
            )
            t1 = wp.tile([128, G * TCH], F32, tag="t1")
            nc.vector.tensor_scalar(
                t1[:, :W], s_w[:, :W], 1.0, 25.0, op0=Alu.mult, op1=Alu.add
            )
            nc.gpsimd.tensor_tensor(t1[:, :W], t1[:, :W], maskf[:, :W], op=Alu.mult)
            nc.gpsimd.tensor_tensor(t1[:, :W], t1[:, :W], cidx[:, :W], op=Alu.add)
            for l, ci in enumerate(cis):
                # per-chunk tidx tile: ap_gather needs an aligned index base
                tidx = wp.tile([128, TCH], I16, tag=f"tidx{l}")
                nc.vector.tensor_tensor(
                    tidx[:],
                    t1[:, l * TCH : (l + 1) * TCH],
                    offs_t[:, 0:TCH],
                    op=Alu.add,
                )
                nc.gpsimd.ap_gather(
                    st_tplg[:, ci * (TCH * 16 * K2) : (ci + 1) * (TCH * 16 * K2)],
                    w1ts[l][:],
                    tidx[:],
                    channels=128,
                    num_elems=TCH * NCAND,
                    d=K2,
                    num_idxs=16 * TCH,
                )
            if not sim_safe:
                tpl_extract_dmas(cis[0], cis[-1] + 1)
            if cis[-1] + 1 == 12:
                flush_outputs(0, 12 * TCH)

        def flush_outputs(q0, q1):
            # staged vec/msk/mcv for per-partition pixel range [q0, q1)
            nc.sync.dma_start(
                o_vec[0:MAIN, :].rearrange("(p q) d -> p q d", p=128)[:, q0:q1],
                st_vec[:, q0 * 2 : q1 * 2],
            )
            nc.sync.dma_start(
                o_msk[0:MAIN, :].rearrange("(p q) one -> p q one", p=128)[:, q0:q1],
                st_msk[:, q0:q1],
            )
            nc.sync.dma_start(
                o_mcv[0:MAIN, :].rearrange("(p q) one -> p q one", p=128)[:, q0:q1],
                st_mcv[:, q0:q1],
            )

        def tpl_extract_dmas(c0, c1):
            # partition-strided extraction for chunks [c0, c1)
            o6 = o_tpl[0:MAIN, :].rearrange(
                "(q j c t) k -> q j c t k", j=16, c=NCHUNK, t=TCH
            )
            for j in range(16):
                sb = st_tplg[j::16].rearrange(
                    "q (c t i k) -> q c t i k", c=NCHUNK, t=TCH, i=16
                )[:, c0:c1, :, j : j + 1, :]
                eng = nc.sync if j % 2 == 0 else nc.scalar
                eng.dma_start(o6[:, j : j + 1, c0:c1], sb)

        # tail first so it overlaps the main stream: 128 pixels
        # [TAIL_OFF, PPC), one per partition
        tl_vec = sp.tile([128, 2], F16, tag="tl_vec")
        tl_tpl = sp.tile([128, K2], F16, tag="tl_tpl")
        tl_msk = sp.tile([128, 1], U8, tag="tl_msk")
        tl_mcv = sp.tile([128, 1], I32, tag="tl_mcv")
        chunk(
            1,
            w1[TAIL_OFF:PPC, :],
            w2[TAIL_OFF:PPC, :],
            tl_vec[:].rearrange("p (t d) -> p t d", t=1),
            tl_tpl[:].rearrange("p (t k) -> p t k", t=1),
            tl_msk[:],
            tl_mcv[:],
        )

        # staggered pipeline: group g's scalar chain + gathers are emitted
        # under group g+1's loads/subs/reduces
        groups = []
        _g = 0
        while _g < NCHUNK:
            n = G if NCHUNK - _g > 3 else (2 if NCHUNK - _g == 3 else NCHUNK - _g)
            groups.append(list(range(_g, _g + n)))
            _g += n
        prev = None
        for cis in groups:
            st = part_load(cis)
            if prev is not None:
                part_chain(prev)
            prev = st
        part_chain(prev)

        flush_outputs(12 * TCH, TPART)
        # template extraction: partition p's valid gather column is i == p%16
        if sim_safe:
            # DVE one-hot extract into dense staging (CoreSim can't check the
            # partition-strided DMAs below)
            st_tpl = sp.tile([128, TPART * K2], F16, tag="st_tpl")
            imkk_t = cpool.tile([128, 16 * K2], F16, tag="imkk_t")
            nc.sync.dma_start(imkk_t[:], imaskk_d)
            for ci in range(NCHUNK):
                sl = st_tplg[:, ci * (TCH * 16 * K2) : (ci + 1) * (TCH * 16 * K2)]
                tp = wp.tile([128, TCH * 16 * K2], F16, tag="tp_ext")
                nc.vector.tensor_tensor(
                    tp[:].rearrange("p (t i k) -> p t i k", i=16, k=K2),
                    sl.rearrange("p (t i k) -> p t i k", i=16, k=K2),
                    imkk_t[:]
                    .rearrange("p (i k) -> p i k", k=K2)
                    .unsqueeze(1)
                    .broadcast_to([128, TCH, 16, K2]),
                    op=Alu.mult,
                )
                with nc.allow_low_precision("one-hot extract"):
                    nc.vector.tensor_reduce(
                        st_tpl[
                            :, ci * TCH * K2 : (ci + 1) * TCH * K2
                        ].rearrange("p (t k) -> p t k", k=K2),
                        tp[:].rearrange("p (t i k) -> p t k i", i=16, k=K2),
                        axis=AX.X,
                        op=Alu.add,
                    )
            nc.sync.dma_start(
                o_tpl[0:MAIN, :].rearrange("(p q) k -> p (q k)", p=128), st_tpl[:]
            )
        nc.sync.dma_start(o_vec[TAIL_OFF:PPC, :], tl_vec[:])
        nc.sync.dma_start(o_tpl[TAIL_OFF:PPC, :], tl_tpl[:])
        nc.sync.dma_start(o_msk[TAIL_OFF:PPC, :], tl_msk[:])
        nc.sync.dma_start(o_mcv[TAIL_OFF:PPC, :], tl_mcv[:])

    return nc


_CACHE = {}


def get_nc(sim_safe=False):
    key = ("nc", sim_safe)
    if key not in _CACHE:
        nc = bacc.Bacc("TRN2", target_bir_lowering=False, debug=False)
        build_program(nc, sim_safe=sim_safe)
        nc.compile()
        _CACHE[key] = nc
    return _CACHE[key]


def make_in_maps(w1, w2):
    w1 = np.ascontiguousarray(np.asarray(w1, dtype=np.float16).reshape(NPIX, E))
    w2 = np.ascontiguousarray(np.asarray(w2, dtype=np.float16).reshape(NPIX, E))
    rank_in = np.ascontiguousarray(np.broadcast_to(RANK64, (128, NCAND)))
    vec_in = np.ascontiguousarray(
        np.broadcast_to(VEC_DC.reshape(-1), (128, 2 * N2))
    )
    rkey_in = np.ascontiguousarray(np.broadcast_to(RKEY2, (128, NCAND)))
    vecrk_in = np.ascontiguousarray(
        np.broadcast_to(VEC_RANKC.reshape(-1), (128, NRANKC * 2))
    )
    imask_in = np.ascontiguousarray(IMASK2)
    offs_in = np.ascontiguousarray(np.broadcast_to(OFFS50, (128, TCH)))
    in_maps = []
    for c in range(NCORES):
        sl = slice(c * PPC, (c + 1) * PPC)
        in_maps.append(
            {
                "w1": np.ascontiguousarray(w1[sl]),
                "w2": np.ascontiguousarray(w2[sl]),
                "rank50": rank_in,
                "vecdc": vec_in,
                "rkey2": rkey_in,
                "vecrk": vecrk_in,
                "imask2": imask_in,
                "imaskk": np.ascontiguousarray(IMASKK),
                "offs50": offs_in,
            }
        )
    return in_maps


def assemble(results):
    vec = np.concatenate([results[c]["o_vec"] for c in range(NCORES)])
    tpl = np.concatenate([results[c]["o_tpl"] for c in range(NCORES)])
    msk = np.concatenate([results[c]["o_msk"] for c in range(NCORES)])
    mcv = np.concatenate([results[c]["o_mcv"] for c in range(NCORES)])
    return (
        vec.reshape(B, H, W, 2).astype(np.float16),
        tpl.reshape(B, H, W, 1, K2).astype(np.float16),
        msk.reshape(B, H, W, 1).astype(bool),
        mcv.reshape(B, H, W, 1).astype(np.int32),
    )


def kernel(w1, w2):
    nc = get_nc()
    in_maps = make_in_maps(w1, w2)
    res = run_bass_kernel_spmd(nc, in_maps, list(range(NCORES)))
    return assemble(res.results)


if __name__ == "__main__":
    rng = np.random.default_rng(0)
    w1 = rng.integers(0, 256, (B, H, W, NCAND, K2)).astype(np.float16)
    w2 = rng.integers(0, 256, (B, H, W, NCAND, K2)).astype(np.float16)
    outs = kernel(w1=w1, w2=w2)
    for o in outs:
        print(o.shape, o.dtype)


# revision 36
# speedup vs baseline: 1.3402x; 1.2941x over previous
"""Trainium2 Bass kernel: block-match SAD cost volume + spiral-tie-break argmin.

Problem (nn_CalculateVector): inputs w1, w2 [1,270,480,50,16] f16 (integer
values 0..255).  Per pixel: SAD cost over K2=16 for 50 candidates, argmin
over the 49 block-match candidates with center-out-spiral tie-break, then
input-MV override, LUT vector output, and template gather.

Strategy: fully data-parallel over the 129600 pixels, 16200 per NeuronCore.
Within a core, pixels are laid out partition-major (partition p owns 126
consecutive pixels) and processed in chunks of TCH pixels per partition.

The spiral argmin is computed exactly with a fused key: key[c] = cost[c] +
RANK[c]/64 (RANK = spiral rank, unique per candidate; cost is an integer
<= 4080 so the fp32 key is exact and min(key) implements first-occurrence-
in-spiral-order argmin).  The input-MV candidate gets rank 63 which makes
min/compare against it implement the reference's strict `<` semantics.

Gathers (VEC_LUT[argmin], w1[argmin]) are computed as one-hot dot products
on the vector engine (eq = key == minkey is exactly one-hot since keys are
unique within a pixel).
"""

import numpy as np

import concourse.bass as bass
import concourse.tile as tile
from concourse import bacc, mybir
from concourse.bass_utils import run_bass_kernel_spmd

SR = 3
NSIDE = 2 * SR + 1
N2 = NSIDE * NSIDE           # 49
K2 = 16
NCAND = N2 + 1               # 50
E = NCAND * K2               # 800 elements per pixel
B, H, W = 1, 270, 480
NPIX = B * H * W             # 129600
NCORES = 8
PPC = NPIX // NCORES         # 16200
TPART = PPC // 128           # 126
MAIN = TPART * 128           # 16128
TAIL_OFF = PPC - 128         # 16072 (tail tile overlaps main region; same values)
TCH = 6                      # pixels per partition per chunk
NCHUNK = TPART // TCH        # 21
G = 3                        # chunks per batched scalar-chain group

F16 = mybir.dt.float16
F32 = mybir.dt.float32
I32 = mybir.dt.int32
U8 = mybir.dt.uint8
Alu = mybir.AluOpType
AX = mybir.AxisListType


def _spiral_order(sz):
    n = 2 * sz + 1
    i = j = 0
    order = [(j + sz) * n + (i + sz)]
    dirs = [(1, 0), (0, 1), (-1, 0), (0, -1)]
    d, step = 0, 1
    while len(order) < n * n:
        for _ in range(2):
            di, dj = dirs[d]
            for _ in range(step):
                i += di
                j += dj
                if abs(i) <= sz and abs(j) <= sz:
                    order.append((j + sz) * n + (i + sz))
            d = (d + 1) % 4
        step += 1
    return np.asarray(order, dtype=np.int32)


def _vec_lut(sz):
    rng = np.arange(-sz, sz + 1)
    jj, ii = np.meshgrid(rng, rng, indexing="ij")
    return (-1.0 * np.stack([jj, ii], axis=-1).reshape(-1, 2)).astype(np.float16)


SPIRAL = _spiral_order(SR)                       # [49]
RANK = np.empty(N2, np.int64)
RANK[SPIRAL] = np.arange(N2)                     # inverse permutation
VEC = _vec_lut(SR)                               # [49, 2] f16, original order
RANK64 = np.zeros(NCAND, np.float32)
RANK64[:N2] = RANK / 64.0
RANK64[N2] = 63.0 / 64.0
VEC_DC = np.ascontiguousarray(VEC.T)             # [2, 49] (d, c) layout

# v2 encoding: key[c] = cost[c] + (RANK[c]*64 + c)/4096 (exact in fp32;
# ordering by RANK dominates so min() still tie-breaks by spiral rank, and
# frac*4096 = RANK*64 + c recovers the winning candidate).  MV candidate gets
# frac 4032/4096 = 63*64/4096 which preserves the strict-< mask semantics.
NRANKC = N2 * 64 + 64                            # 3200 table entries (max idx 3120)
RKEY2 = np.zeros(NCAND, np.float32)
RKEY2[:N2] = (RANK * 64 + np.arange(N2)) / 4096.0
RKEY2[N2] = 4032.0 / 4096.0
VEC_RANKC = np.zeros((NRANKC, 2), np.float16)    # rankc -> VEC[c]
for _c in range(N2):
    VEC_RANKC[RANK[_c] * 64 + _c] = VEC[_c]
IMASK2 = np.zeros((128, 16 * 2), np.float16)     # diag extract: i == p%16
for _p in range(128):
    IMASK2[_p, (_p % 16) * 2 : (_p % 16) * 2 + 2] = 1.0
IMASKK = np.zeros((128, 16 * K2), np.float16)    # same at K2 granularity
for _p in range(128):
    IMASKK[_p, (_p % 16) * K2 : (_p % 16 + 1) * K2] = 1.0
OFFS50 = np.tile(np.arange(TCH, dtype=np.float32) * NCAND, G)  # + t*50 per chunk


I16 = mybir.dt.int16


def build_program(nc, sim_safe=False):
    w1 = nc.dram_tensor("w1", [PPC, E], F16, kind="ExternalInput").ap()
    w2 = nc.dram_tensor("w2", [PPC, E], F16, kind="ExternalInput").ap()
    rank_d = nc.dram_tensor("rank50", [128, NCAND], F32, kind="ExternalInput").ap()
    vec_d = nc.dram_tensor("vecdc", [128, 2 * N2], F16, kind="ExternalInput").ap()
    rkey_d = nc.dram_tensor("rkey2", [128, NCAND], F32, kind="ExternalInput").ap()
    vecrk_d = nc.dram_tensor(
        "vecrk", [128, NRANKC * 2], F16, kind="ExternalInput"
    ).ap()
    imask_d = nc.dram_tensor("imask2", [128, 32], F16, kind="ExternalInput").ap()
    imaskk_d = nc.dram_tensor(
        "imaskk", [128, 16 * K2], F16, kind="ExternalInput"
    ).ap()
    offs_d = nc.dram_tensor(
        "offs50", [128, G * TCH], F32, kind="ExternalInput"
    ).ap()
    o_vec = nc.dram_tensor("o_vec", [PPC, 2], F16, kind="ExternalOutput").ap()
    o_tpl = nc.dram_tensor("o_tpl", [PPC, K2], F16, kind="ExternalOutput").ap()
    o_msk = nc.dram_tensor("o_msk", [PPC, 1], U8, kind="ExternalOutput").ap()
    o_mcv = nc.dram_tensor("o_mcv", [PPC, 1], I32, kind="ExternalOutput").ap()

    from contextlib import ExitStack

    with tile.TileContext(nc) as tc, ExitStack() as ctx:
        cpool = ctx.enter_context(tc.tile_pool(name="const", bufs=1))
        w1p = ctx.enter_context(
            tc.tile_pool(name="w1p", bufs=(4 if sim_safe else 2 * G + 1))
        )
        w2p = ctx.enter_context(tc.tile_pool(name="w2p", bufs=3))
        wp = ctx.enter_context(tc.tile_pool(name="work", bufs=2))
        sp = ctx.enter_context(tc.tile_pool(name="stage", bufs=1))

        rank_t = cpool.tile([128, NCAND], F32, tag="rank_t")
        nc.sync.dma_start(rank_t[:], rank_d)
        vec_t = cpool.tile([128, 2 * N2], F16, tag="vec_t")
        nc.sync.dma_start(vec_t[:], vec_d)
        rkey_t = cpool.tile([128, NCAND], F32, tag="rkey_t")
        nc.sync.dma_start(rkey_t[:], rkey_d)
        vecrk_t = cpool.tile([128, NRANKC * 2], F16, tag="vecrk_t")
        nc.sync.dma_start(vecrk_t[:], vecrk_d)
        imask_t = cpool.tile([128, 32], F16, tag="imask_t")
        nc.sync.dma_start(imask_t[:], imask_d)
        offs_t = cpool.tile([128, TCH], F32, tag="offs_t")
        nc.sync.dma_start(offs_t[:], offs_d)

        st_vec = sp.tile([128, TPART * 2], F16, tag="st_vec")
        st_msk = sp.tile([128, TPART], U8, tag="st_msk")
        st_mcv = sp.tile([128, TPART], I32, tag="st_mcv")
        # gathered templates, (chunk, t, i, k) per partition; valid at i==p%16
        st_tplg = sp.tile([128, NCHUNK * TCH * 16 * K2], F16, tag="st_tplg")

        w1m = w1[0:MAIN, :].rearrange("(p q) e -> p (q e)", p=128)
        w2m = w2[0:MAIN, :].rearrange("(p q) e -> p (q e)", p=128)

        def chunk(T, src1, src2, ovec, otpl, omsk, omcv):
            # v1 path (one-hot dot products on DVE) — used for the tail only.
            w1t = w1p.tile([128, T * E], F16, tag="w1t")
            nc.sync.dma_start(w1t[:], src1)
            w2t = w2p.tile([128, T * E], F16, tag="w2t")
            nc.scalar.dma_start(w2t[:], src2)

            diff = wp.tile([128, T * E], F16, tag="diff_v1")
            nc.vector.tensor_tensor(diff[:], w1t[:], w2t[:], op=Alu.subtract)

            cost = wp.tile([128, T * NCAND], F32, tag="cost_v1")
            cost3 = cost[:].rearrange("p (t c) -> p t c", c=NCAND)
            nc.vector.tensor_reduce(
                cost3,
                diff[:].rearrange("p (t c k) -> p t c k", c=NCAND, k=K2),
                axis=AX.X,
                op=Alu.add,
                apply_absolute_value=True,
            )

            key = wp.tile([128, T * NCAND], F32, tag="key_v1")
            key3 = key[:].rearrange("p (t c) -> p t c", c=NCAND)
            nc.vector.tensor_tensor(
                key3,
                cost3,
                rank_t[:].unsqueeze(1).broadcast_to([128, T, NCAND]),
                op=Alu.add,
            )

            minkbm = wp.tile([128, T], F32, tag="minkbm_v1")
            nc.vector.tensor_reduce(
                minkbm[:], key3[:, :, 0:N2], axis=AX.X, op=Alu.min
            )

            kmv = key3[:, :, N2:NCAND].rearrange("p t one -> p (t one)")
            mk50 = wp.tile([128, T], F32, tag="mk50_v1")
            nc.vector.tensor_tensor(mk50[:], kmv, minkbm[:], op=Alu.min)

            maskf = wp.tile([128, T], F32, tag="maskf_v1")
            nc.vector.tensor_tensor(maskf[:], kmv, minkbm[:], op=Alu.is_lt)
            nc.scalar.copy(omsk, maskf[:])

            nc.vector.tensor_reduce(omcv, cost3, axis=AX.X, op=Alu.min)

            keyct = key[:].rearrange("p (t c) -> p c t", c=NCAND)
            eq50 = wp.tile([128, NCAND * T], F16, tag="eq50_v1")
            eq50v = eq50[:].rearrange("p (c t) -> p c t", t=T)
            nc.vector.tensor_tensor(
                eq50v,
                keyct,
                mk50[:].unsqueeze(1).broadcast_to([128, NCAND, T]),
                op=Alu.is_equal,
            )
            eq49 = wp.tile([128, N2 * T], F16, tag="eq49_v1")
            eq49v = eq49[:].rearrange("p (c t) -> p c t", t=T)
            nc.vector.tensor_tensor(
                eq49v,
                keyct[:, 0:N2, :],
                minkbm[:].unsqueeze(1).broadcast_to([128, N2, T]),
                op=Alu.is_equal,
            )

            prodv = wp.tile([128, T * 2 * N2], F16, tag="prodv_v1")
            prodv4 = prodv[:].rearrange("p (t d c) -> p t d c", d=2, c=N2)
            e49_tdc = (
                eq49[:]
                .rearrange("p (c t) -> p t c", t=T)
                .unsqueeze(2)
                .broadcast_to([128, T, 2, N2])
            )
            vdc = (
                vec_t[:]
                .rearrange("p (d c) -> p d c", c=N2)
                .unsqueeze(1)
                .broadcast_to([128, T, 2, N2])
            )
            nc.vector.tensor_tensor(prodv4, e49_tdc, vdc, op=Alu.mult)
            with nc.allow_low_precision("one-hot dot, sums are exact"):
                nc.vector.tensor_reduce(ovec, prodv4, axis=AX.X, op=Alu.add)

            prod8 = wp.tile([128, T * K2 * NCAND], F16, tag="prod8_v1")
            prod84 = prod8[:].rearrange("p (t k c) -> p t k c", k=K2, c=NCAND)
            w1_tkc = w1t[:].rearrange("p (t c k) -> p t k c", c=NCAND, k=K2)
            eq50_tkc = (
                eq50[:]
                .rearrange("p (c t) -> p t c", t=T)
                .unsqueeze(2)
                .broadcast_to([128, T, K2, NCAND])
            )
            nc.vector.tensor_tensor(prod84, w1_tkc, eq50_tkc, op=Alu.mult)
            with nc.allow_low_precision("one-hot dot, sums are exact"):
                nc.vector.tensor_reduce(otpl, prod84, axis=AX.X, op=Alu.add)

        # ---- v3 main path: per-chunk streaming (sub+reduce) with the scalar
        # chain batched per group of G chunks, GPSIMD doing gathers and the
        # small tensor-tensor chain, and partition-strided extraction DMAs.
        def part_load(cis):
            # loads + subtract + grouped SAD reduce for a group of chunks
            w1ts = []
            cost_w = wp.tile([128, G * TCH * NCAND], F32, tag="cost_w")
            for l, ci in enumerate(cis):
                t0 = ci * TCH
                sl = slice(t0 * E, (t0 + TCH) * E)
                if ci % 2 == 0:
                    eng1, eng2 = nc.sync, nc.scalar
                else:
                    eng1, eng2 = nc.scalar, nc.sync
                w1t = w1p.tile([128, TCH * E], F16, tag="w1t")
                eng1.dma_start(w1t[:], w1m[:, sl])
                w2t = w2p.tile([128, TCH * E], F16, tag="w2t")
                eng2.dma_start(w2t[:], w2m[:, sl])
                w1ts.append(w1t)
                # diff in place of w2t (w2 is dead after this)
                nc.vector.tensor_tensor(w2t[:], w1t[:], w2t[:], op=Alu.subtract)
                nc.vector.tensor_reduce(
                    cost_w[:, l * TCH * NCAND : (l + 1) * TCH * NCAND].rearrange(
                        "p (t c) -> p t c", c=NCAND
                    ),
                    w2t[:].rearrange("p (t c k) -> p t c k", c=NCAND, k=K2),
                    axis=AX.X,
                    op=Alu.add,
                    apply_absolute_value=True,
                )
            return {"cis": cis, "w1ts": w1ts, "cost_w": cost_w}

        def part_chain(st):
            cis, w1ts, cost_w = st["cis"], st["w1ts"], st["cost_w"]
            W = len(cis) * TCH
            cost3 = cost_w[:, : W * NCAND].rearrange("p (t c) -> p t c", c=NCAND)
            key_w = wp.tile([128, G * TCH * NCAND], F32, tag="key_w")
            key3 = key_w[:, : W * NCAND].rearrange("p (t c) -> p t c", c=NCAND)
            nc.vector.tensor_tensor(
                key3,
                cost3,
                rkey_t[:].unsqueeze(1).broadcast_to([128, W, NCAND]),
                op=Alu.add,
            )
            minkbm = wp.tile([128, G * TCH], F32, tag="minkbm")
            nc.vector.tensor_reduce(
                minkbm[:, :W], key3[:, :, 0:N2], axis=AX.X, op=Alu.min
            )
            mincbm = wp.tile([128, G * TCH], F32, tag="mincbm")
            nc.vector.tensor_reduce(
                mincbm[:, :W], cost3[:, :, 0:N2], axis=AX.X, op=Alu.min
            )
            kmv = key3[:, :, N2:NCAND].rearrange("p t one -> p (t one)")
            cmv = cost3[:, :, N2:NCAND].rearrange("p t one -> p (t one)")
            maskf = wp.tile([128, G * TCH], F32, tag="maskf")
            nc.vector.tensor_tensor(maskf[:, :W], kmv, minkbm[:, :W], op=Alu.is_lt)
            p0 = cis[0] * TCH
            nc.scalar.copy(st_msk[:, p0 : p0 + W], maskf[:, :W])
            nc.vector.tensor_tensor(
                st_mcv[:, p0 : p0 + W], cmv, mincbm[:, :W], op=Alu.min
            )
            frac = wp.tile([128, G * TCH], F32, tag="frac")
            nc.vector.tensor_tensor(
                frac[:, :W], minkbm[:, :W], mincbm[:, :W], op=Alu.subtract
            )
            rci = wp.tile([128, G * TCH], I16, tag="rci")
            nc.vector.tensor_scalar(
                rci[:, :W], frac[:, :W], 4096.0, None, op0=Alu.mult
            )
            gv = wp.tile([128, G * TCH * 32], F16, tag="gv")
            nc.gpsimd.ap_gather(
                gv[:, : W * 32],
                vecrk_t[:],
                rci[:, :W],
                channels=128,
                num_elems=NRANKC,
                d=2,
                num_idxs=16 * W,
            )
            vprod = wp.tile([128, G * TCH * 32], F16, tag="vprod")
            nc.vector.tensor_tensor(
                vprod[:, : W * 32].rearrange("p (t i d) -> p t i d", i=16, d=2),
                gv[:, : W * 32].rearrange("p (t i d) -> p t i d", i=16, d=2),
                imask_t[:]
                .rearrange("p (i d) -> p i d", d=2)
                .unsqueeze(1)
                .broadcast_to([128, W, 16, 2]),
                op=Alu.mult,
            )
            ovec = st_vec[:, p0 * 2 : (p0 + W) * 2].rearrange(
                "p (t d) -> p t d", d=2
            )
            with nc.allow_low_precision("one-hot extract, sums exact"):
                nc.vector.tensor_reduce(
                    ovec,
                    vprod[:, : W * 32].rearrange("p (t i d) -> p t d i", i=16, d=2),
                    axis=AX.X,
                    op=Alu.add,
                )
            vsl = st_vec[:, p0 * 2 : (p0 + W) * 2].rearrange("p (t d) -> p t d", d=2)
            v0 = vsl[:, :, 0:1].rearrange("p t one -> p (t one)")
            v1 = vsl[:, :, 1:2].rearrange("p t one -> p (t one)")
            # s = 7*v0 + v1; c = 24 - s; 49 - c = 25 + s
            s_w = wp.tile([128, G * TCH], F32, tag="s_w")
            nc.vector.scalar_tensor_tensor(
                s_w[:, :W], v0, 7.0, v1, op0=Alu.mult, op1=Alu.add
            )
            cidx = wp.tile([128, G * TCH], F32, tag="cidx")
            nc.vector.tensor_scalar(
                cidx[:, :W], s_w[:, :W], -1.0, 24.0, op0=Alu.mult, op1=Alu.add
            )
            t1 = wp.tile([128, G * TCH], F32, tag="t1")
            nc.vector.tensor_scalar(
                t1[:, :W], s_w[:, :W], 1.0, 25.0, op0=Alu.mult, op1=Alu.add
            )
            nc.vector.tensor_tensor(t1[:, :W], t1[:, :W], maskf[:, :W], op=Alu.mult)
            nc.vector.tensor_tensor(t1[:, :W], t1[:, :W], cidx[:, :W], op=Alu.add)
            for l, ci in enumerate(cis):
                # per-chunk tidx tile: ap_gather needs an aligned index base
                tidx = wp.tile([128, TCH], I16, tag=f"tidx{l}")
                nc.vector.tensor_tensor(
                    tidx[:],
                    t1[:, l * TCH : (l + 1) * TCH],
                    offs_t[:, 0:TCH],
                    op=Alu.add,
                )
                nc.gpsimd.ap_gather(
                    st_tplg[:, ci * (TCH * 16 * K2) : (ci + 1) * (TCH * 16 * K2)],
                    w1ts[l][:],
                    tidx[:],
                    channels=128,
                    num_elems=TCH * NCAND,
                    d=K2,
                    num_idxs=16 * TCH,
                )
            if not sim_safe:
                tpl_extract_dmas(cis[0], cis[-1] + 1)
            if cis[-1] + 1 == 12:
                flush_outputs(0, 12 * TCH)

        def flush_outputs(q0, q1):
            # staged vec/msk/mcv for per-partition pixel range [q0, q1)
            nc.sync.dma_start(
                o_vec[0:MAIN, :].rearrange("(p q) d -> p q d", p=128)[:, q0:q1],
                st_vec[:, q0 * 2 : q1 * 2],
            )
            nc.sync.dma_start(
                o_msk[0:MAIN, :].rearrange("(p q) one -> p q one", p=128)[:, q0:q1],
                st_msk[:, q0:q1],
            )
            nc.sync.dma_start(
                o_mcv[0:MAIN, :].rearrange("(p q) one -> p q one", p=128)[:, q0:q1],
                st_mcv[:, q0:q1],
            )

        def tpl_extract_dmas(c0, c1):
            # partition-strided extraction for chunks [c0, c1)
            o6 = o_tpl[0:MAIN, :].rearrange(
                "(q j c t) k -> q j c t k", j=16, c=NCHUNK, t=TCH
            )
            for j in range(16):
                sb = st_tplg[j::16].rearrange(
                    "q (c t i k) -> q c t i k", c=NCHUNK, t=TCH, i=16
                )[:, c0:c1, :, j : j + 1, :]
                eng = nc.sync if j % 2 == 0 else nc.scalar
                eng.dma_start(o6[:, j : j + 1, c0:c1], sb)

        # tail first so it overlaps the main stream: 128 pixels
        # [TAIL_OFF, PPC), one per partition
        tl_vec = sp.tile([128, 2], F16, tag="tl_vec")
        tl_tpl = sp.tile([128, K2], F16, tag="tl_tpl")
        tl_msk = sp.tile([128, 1], U8, tag="tl_msk")
        tl_mcv = sp.tile([128, 1], I32, tag="tl_mcv")
        chunk(
            1,
            w1[TAIL_OFF:PPC, :],
            w2[TAIL_OFF:PPC, :],
            tl_vec[:].rearrange("p (t d) -> p t d", t=1),
            tl_tpl[:].rearrange("p (t k) -> p t k", t=1),
            tl_msk[:],
            tl_mcv[:],
        )

        # staggered pipeline: group g's scalar chain + gathers are emitted
        # under group g+1's loads/subs/reduces
        groups = []
        _g = 0
        while _g < NCHUNK:
            n = G if NCHUNK - _g > 3 else (2 if NCHUNK - _g == 3 else NCHUNK - _g)
            groups.append(list(range(_g, _g + n)))
            _g += n
        prev = None
        for cis in groups:
            st = part_load(cis)
            if prev is not None:
                part_chain(prev)
            prev = st
        part_chain(prev)

        flush_outputs(12 * TCH, TPART)
        # template extraction: partition p's valid gather column is i == p%16
        if sim_safe:
            # DVE one-hot extract into dense staging (CoreSim can't check the
            # partition-strided DMAs below)
            st_tpl = sp.tile([128, TPART * K2], F16, tag="st_tpl")
            imkk_t = cpool.tile([128, 16 * K2], F16, tag="imkk_t")
            nc.sync.dma_start(imkk_t[:], imaskk_d)
            for ci in range(NCHUNK):
                sl = st_tplg[:, ci * (TCH * 16 * K2) : (ci + 1) * (TCH * 16 * K2)]
                tp = wp.tile([128, TCH * 16 * K2], F16, tag="tp_ext")
                nc.vector.tensor_tensor(
                    tp[:].rearrange("p (t i k) -> p t i k", i=16, k=K2),
                    sl.rearrange("p (t i k) -> p t i k", i=16, k=K2),
                    imkk_t[:]
                    .rearrange("p (i k) -> p i k", k=K2)
                    .unsqueeze(1)
                    .broadcast_to([128, TCH, 16, K2]),
                    op=Alu.mult,
                )
                with nc.allow_low_precision("one-hot extract"):
                    nc.vector.tensor_reduce(
                        st_tpl[
                            :, ci * TCH * K2 : (ci + 1) * TCH * K2
                        ].rearrange("p (t k) -> p t k", k=K2),
                        tp[:].rearrange("p (t i k) -> p t k i", i=16, k=K2),
                        axis=AX.X,
                        op=Alu.add,
                    )
            nc.sync.dma_start(
                o_tpl[0:MAIN, :].rearrange("(p q) k -> p (q k)", p=128), st_tpl[:]
            )
        nc.sync.dma_start(o_vec[TAIL_OFF:PPC, :], tl_vec[:])
        nc.sync.dma_start(o_tpl[TAIL_OFF:PPC, :], tl_tpl[:])
        nc.sync.dma_start(o_msk[TAIL_OFF:PPC, :], tl_msk[:])
        nc.sync.dma_start(o_mcv[TAIL_OFF:PPC, :], tl_mcv[:])

    return nc


_CACHE = {}


def get_nc(sim_safe=False):
    key = ("nc", sim_safe)
    if key not in _CACHE:
        nc = bacc.Bacc("TRN2", target_bir_lowering=False, debug=False)
        build_program(nc, sim_safe=sim_safe)
        nc.compile()
        _CACHE[key] = nc
    return _CACHE[key]


def make_in_maps(w1, w2):
    w1 = np.ascontiguousarray(np.asarray(w1, dtype=np.float16).reshape(NPIX, E))
    w2 = np.ascontiguousarray(np.asarray(w2, dtype=np.float16).reshape(NPIX, E))
    rank_in = np.ascontiguousarray(np.broadcast_to(RANK64, (128, NCAND)))
    vec_in = np.ascontiguousarray(
        np.broadcast_to(VEC_DC.reshape(-1), (128, 2 * N2))
    )
    rkey_in = np.ascontiguousarray(np.broadcast_to(RKEY2, (128, NCAND)))
    vecrk_in = np.ascontiguousarray(
        np.broadcast_to(VEC_RANKC.reshape(-1), (128, NRANKC * 2))
    )
    imask_in = np.ascontiguousarray(IMASK2)
    offs_in = np.ascontiguousarray(np.broadcast_to(OFFS50, (128, TCH)))
    in_maps = []
    for c in range(NCORES):
        sl = slice(c * PPC, (c + 1) * PPC)
        in_maps.append(
            {
                "w1": np.ascontiguousarray(w1[sl]),
                "w2": np.ascontiguousarray(w2[sl]),
                "rank50": rank_in,
                "vecdc": vec_in,
                "rkey2": rkey_in,
                "vecrk": vecrk_in,
                "imask2": imask_in,
                "imaskk": np.ascontiguousarray(IMASKK),
                "offs50": offs_in,
            }
        )
    return in_maps


def assemble(results):
    vec = np.concatenate([results[c]["o_vec"] for c in range(NCORES)])
    tpl = np.concatenate([results[c]["o_tpl"] for c in range(NCORES)])
    msk = np.concatenate([results[c]["o_msk"] for c in range(NCORES)])
    mcv = np.concatenate([results[c]["o_mcv"] for c in range(NCORES)])
    return (
        vec.reshape(B, H, W, 2).astype(np.float16),
        tpl.reshape(B, H, W, 1, K2).astype(np.float16),
        msk.reshape(B, H, W, 1).astype(bool),
        mcv.reshape(B, H, W, 1).astype(np.int32),
    )


def kernel(w1, w2):
    nc = get_nc()
    in_maps = make_in_maps(w1, w2)
    res = run_bass_kernel_spmd(nc, in_maps, list(range(NCORES)))
    return assemble(res.results)


if __name__ == "__main__":
    rng = np.random.default_rng(0)
    w1 = rng.integers(0, 256, (B, H, W, NCAND, K2)).astype(np.float16)
    w2 = rng.integers(0, 256, (B, H, W, NCAND, K2)).astype(np.float16)
    outs = kernel(w1=w1, w2=w2)
    for o in outs:
        print(o.shape, o.dtype)


# revision 37
# speedup vs baseline: 1.4593x; 1.0889x over previous
"""Trainium2 Bass kernel: block-match SAD cost volume + spiral-tie-break argmin.

Problem (nn_CalculateVector): inputs w1, w2 [1,270,480,50,16] f16 (integer
values 0..255).  Per pixel: SAD cost over K2=16 for 50 candidates, argmin
over the 49 block-match candidates with center-out-spiral tie-break, then
input-MV override, LUT vector output, and template gather.

Strategy: fully data-parallel over the 129600 pixels, 16200 per NeuronCore.
Within a core, pixels are laid out partition-major (partition p owns 126
consecutive pixels) and processed in chunks of TCH pixels per partition.

The spiral argmin is computed exactly with a fused key: key[c] = cost[c] +
RANK[c]/64 (RANK = spiral rank, unique per candidate; cost is an integer
<= 4080 so the fp32 key is exact and min(key) implements first-occurrence-
in-spiral-order argmin).  The input-MV candidate gets rank 63 which makes
min/compare against it implement the reference's strict `<` semantics.

Gathers (VEC_LUT[argmin], w1[argmin]) are computed as one-hot dot products
on the vector engine (eq = key == minkey is exactly one-hot since keys are
unique within a pixel).
"""

import numpy as np

import concourse.bass as bass
import concourse.tile as tile
from concourse import bacc, mybir
from concourse.bass_utils import run_bass_kernel_spmd

SR = 3
NSIDE = 2 * SR + 1
N2 = NSIDE * NSIDE           # 49
K2 = 16
NCAND = N2 + 1               # 50
E = NCAND * K2               # 800 elements per pixel
B, H, W = 1, 270, 480
NPIX = B * H * W             # 129600
NCORES = 8
PPC = NPIX // NCORES         # 16200
TPART = PPC // 128           # 126
MAIN = TPART * 128           # 16128
TAIL_OFF = PPC - 128         # 16072 (tail tile overlaps main region; same values)
TCH = 6                      # pixels per partition per chunk
NCHUNK = TPART // TCH        # 21
G = 3                        # chunks per batched scalar-chain group

F16 = mybir.dt.float16
F32 = mybir.dt.float32
I32 = mybir.dt.int32
U8 = mybir.dt.uint8
Alu = mybir.AluOpType
AX = mybir.AxisListType


def _spiral_order(sz):
    n = 2 * sz + 1
    i = j = 0
    order = [(j + sz) * n + (i + sz)]
    dirs = [(1, 0), (0, 1), (-1, 0), (0, -1)]
    d, step = 0, 1
    while len(order) < n * n:
        for _ in range(2):
            di, dj = dirs[d]
            for _ in range(step):
                i += di
                j += dj
                if abs(i) <= sz and abs(j) <= sz:
                    order.append((j + sz) * n + (i + sz))
            d = (d + 1) % 4
        step += 1
    return np.asarray(order, dtype=np.int32)


def _vec_lut(sz):
    rng = np.arange(-sz, sz + 1)
    jj, ii = np.meshgrid(rng, rng, indexing="ij")
    return (-1.0 * np.stack([jj, ii], axis=-1).reshape(-1, 2)).astype(np.float16)


SPIRAL = _spiral_order(SR)                       # [49]
RANK = np.empty(N2, np.int64)
RANK[SPIRAL] = np.arange(N2)                     # inverse permutation
VEC = _vec_lut(SR)                               # [49, 2] f16, original order
RANK64 = np.zeros(NCAND, np.float32)
RANK64[:N2] = RANK / 64.0
RANK64[N2] = 63.0 / 64.0
VEC_DC = np.ascontiguousarray(VEC.T)             # [2, 49] (d, c) layout

# v2 encoding: key[c] = cost[c] + (RANK[c]*64 + c)/4096 (exact in fp32;
# ordering by RANK dominates so min() still tie-breaks by spiral rank, and
# frac*4096 = RANK*64 + c recovers the winning candidate).  MV candidate gets
# frac 4032/4096 = 63*64/4096 which preserves the strict-< mask semantics.
NRANKC = N2 * 64 + 64                            # 3200 table entries (max idx 3120)
RKEY2 = np.zeros(NCAND, np.float32)
RKEY2[:N2] = (RANK * 64 + np.arange(N2)) / 4096.0
RKEY2[N2] = 4032.0 / 4096.0
VEC_RANKC = np.zeros((NRANKC, 2), np.float16)    # rankc -> VEC[c]
for _c in range(N2):
    VEC_RANKC[RANK[_c] * 64 + _c] = VEC[_c]
IMASK2 = np.zeros((128, 16 * 2), np.float16)     # diag extract: i == p%16
for _p in range(128):
    IMASK2[_p, (_p % 16) * 2 : (_p % 16) * 2 + 2] = 1.0
IMASKK = np.zeros((128, 16 * K2), np.float16)    # same at K2 granularity
for _p in range(128):
    IMASKK[_p, (_p % 16) * K2 : (_p % 16 + 1) * K2] = 1.0
OFFS50 = np.tile(np.arange(TCH, dtype=np.float32) * NCAND, G)  # + t*50 per chunk


I16 = mybir.dt.int16


def build_program(nc, sim_safe=False):
    w1 = nc.dram_tensor("w1", [PPC, E], F16, kind="ExternalInput").ap()
    w2 = nc.dram_tensor("w2", [PPC, E], F16, kind="ExternalInput").ap()
    rank_d = nc.dram_tensor("rank50", [128, NCAND], F32, kind="ExternalInput").ap()
    vec_d = nc.dram_tensor("vecdc", [128, 2 * N2], F16, kind="ExternalInput").ap()
    rkey_d = nc.dram_tensor("rkey2", [128, NCAND], F32, kind="ExternalInput").ap()
    vecrk_d = nc.dram_tensor(
        "vecrk", [128, NRANKC * 2], F16, kind="ExternalInput"
    ).ap()
    imask_d = nc.dram_tensor("imask2", [128, 32], F16, kind="ExternalInput").ap()
    imaskk_d = nc.dram_tensor(
        "imaskk", [128, 16 * K2], F16, kind="ExternalInput"
    ).ap()
    offs_d = nc.dram_tensor(
        "offs50", [128, G * TCH], F32, kind="ExternalInput"
    ).ap()
    o_vec = nc.dram_tensor("o_vec", [PPC, 2], F16, kind="ExternalOutput").ap()
    o_tpl = nc.dram_tensor("o_tpl", [PPC, K2], F16, kind="ExternalOutput").ap()
    o_msk = nc.dram_tensor("o_msk", [PPC, 1], U8, kind="ExternalOutput").ap()
    o_mcv = nc.dram_tensor("o_mcv", [PPC, 1], I32, kind="ExternalOutput").ap()

    from contextlib import ExitStack

    with tile.TileContext(nc) as tc, ExitStack() as ctx:
        cpool = ctx.enter_context(tc.tile_pool(name="const", bufs=1))
        w1p = ctx.enter_context(
            tc.tile_pool(name="w1p", bufs=(4 if sim_safe else 2 * G + 1))
        )
        w2p = ctx.enter_context(tc.tile_pool(name="w2p", bufs=3))
        wp = ctx.enter_context(tc.tile_pool(name="work", bufs=2))
        sp = ctx.enter_context(tc.tile_pool(name="stage", bufs=1))

        rank_t = cpool.tile([128, NCAND], F32, tag="rank_t")
        nc.sync.dma_start(rank_t[:], rank_d)
        vec_t = cpool.tile([128, 2 * N2], F16, tag="vec_t")
        nc.sync.dma_start(vec_t[:], vec_d)
        rkey_t = cpool.tile([128, NCAND], F32, tag="rkey_t")
        nc.sync.dma_start(rkey_t[:], rkey_d)
        vecrk_t = cpool.tile([128, NRANKC * 2], F16, tag="vecrk_t")
        nc.scalar.dma_start(vecrk_t[:], vecrk_d)
        imask_t = cpool.tile([128, 32], F16, tag="imask_t")
        nc.sync.dma_start(imask_t[:], imask_d)
        offs_t = cpool.tile([128, TCH], F32, tag="offs_t")
        nc.sync.dma_start(offs_t[:], offs_d)

        st_vec = sp.tile([128, TPART * 2], F16, tag="st_vec")
        st_msk = sp.tile([128, TPART], U8, tag="st_msk")
        st_mcv = sp.tile([128, TPART], I32, tag="st_mcv")
        # gathered templates, (chunk, t, i, k) per partition; valid at i==p%16
        st_tplg = sp.tile([128, NCHUNK * TCH * 16 * K2], F16, tag="st_tplg")

        w1m = w1[0:MAIN, :].rearrange("(p q) e -> p (q e)", p=128)
        w2m = w2[0:MAIN, :].rearrange("(p q) e -> p (q e)", p=128)

        def chunk(T, src1, src2, ovec, otpl, omsk, omcv):
            # v1 path (one-hot dot products on DVE) — used for the tail only.
            w1t = w1p.tile([128, T * E], F16, tag="w1t")
            nc.sync.dma_start(w1t[:], src1)
            w2t = w2p.tile([128, T * E], F16, tag="w2t")
            nc.scalar.dma_start(w2t[:], src2)

            diff = wp.tile([128, T * E], F16, tag="diff_v1")
            nc.vector.tensor_tensor(diff[:], w1t[:], w2t[:], op=Alu.subtract)

            cost = wp.tile([128, T * NCAND], F32, tag="cost_v1")
            cost3 = cost[:].rearrange("p (t c) -> p t c", c=NCAND)
            nc.vector.tensor_reduce(
                cost3,
                diff[:].rearrange("p (t c k) -> p t c k", c=NCAND, k=K2),
                axis=AX.X,
                op=Alu.add,
                apply_absolute_value=True,
            )

            key = wp.tile([128, T * NCAND], F32, tag="key_v1")
            key3 = key[:].rearrange("p (t c) -> p t c", c=NCAND)
            nc.vector.tensor_tensor(
                key3,
                cost3,
                rank_t[:].unsqueeze(1).broadcast_to([128, T, NCAND]),
                op=Alu.add,
            )

            minkbm = wp.tile([128, T], F32, tag="minkbm_v1")
            nc.vector.tensor_reduce(
                minkbm[:], key3[:, :, 0:N2], axis=AX.X, op=Alu.min
            )

            kmv = key3[:, :, N2:NCAND].rearrange("p t one -> p (t one)")
            mk50 = wp.tile([128, T], F32, tag="mk50_v1")
            nc.vector.tensor_tensor(mk50[:], kmv, minkbm[:], op=Alu.min)

            maskf = wp.tile([128, T], F32, tag="maskf_v1")
            nc.vector.tensor_tensor(maskf[:], kmv, minkbm[:], op=Alu.is_lt)
            nc.scalar.copy(omsk, maskf[:])

            nc.vector.tensor_reduce(omcv, cost3, axis=AX.X, op=Alu.min)

            keyct = key[:].rearrange("p (t c) -> p c t", c=NCAND)
            eq50 = wp.tile([128, NCAND * T], F16, tag="eq50_v1")
            eq50v = eq50[:].rearrange("p (c t) -> p c t", t=T)
            nc.vector.tensor_tensor(
                eq50v,
                keyct,
                mk50[:].unsqueeze(1).broadcast_to([128, NCAND, T]),
                op=Alu.is_equal,
            )
            eq49 = wp.tile([128, N2 * T], F16, tag="eq49_v1")
            eq49v = eq49[:].rearrange("p (c t) -> p c t", t=T)
            nc.vector.tensor_tensor(
                eq49v,
                keyct[:, 0:N2, :],
                minkbm[:].unsqueeze(1).broadcast_to([128, N2, T]),
                op=Alu.is_equal,
            )

            prodv = wp.tile([128, T * 2 * N2], F16, tag="prodv_v1")
            prodv4 = prodv[:].rearrange("p (t d c) -> p t d c", d=2, c=N2)
            e49_tdc = (
                eq49[:]
                .rearrange("p (c t) -> p t c", t=T)
                .unsqueeze(2)
                .broadcast_to([128, T, 2, N2])
            )
            vdc = (
                vec_t[:]
                .rearrange("p (d c) -> p d c", c=N2)
                .unsqueeze(1)
                .broadcast_to([128, T, 2, N2])
            )
            nc.vector.tensor_tensor(prodv4, e49_tdc, vdc, op=Alu.mult)
            with nc.allow_low_precision("one-hot dot, sums are exact"):
                nc.vector.tensor_reduce(ovec, prodv4, axis=AX.X, op=Alu.add)

            prod8 = wp.tile([128, T * K2 * NCAND], F16, tag="prod8_v1")
            prod84 = prod8[:].rearrange("p (t k c) -> p t k c", k=K2, c=NCAND)
            w1_tkc = w1t[:].rearrange("p (t c k) -> p t k c", c=NCAND, k=K2)
            eq50_tkc = (
                eq50[:]
                .rearrange("p (c t) -> p t c", t=T)
                .unsqueeze(2)
                .broadcast_to([128, T, K2, NCAND])
            )
            nc.vector.tensor_tensor(prod84, w1_tkc, eq50_tkc, op=Alu.mult)
            with nc.allow_low_precision("one-hot dot, sums are exact"):
                nc.vector.tensor_reduce(otpl, prod84, axis=AX.X, op=Alu.add)

        # ---- v3 main path: per-chunk streaming (sub+reduce) with the scalar
        # chain batched per group of G chunks, GPSIMD doing gathers and the
        # small tensor-tensor chain, and partition-strided extraction DMAs.
        def part_load(cis):
            # loads + subtract + grouped SAD reduce for a group of chunks
            w1ts = []
            cost_w = wp.tile([128, G * TCH * NCAND], F32, tag="cost_w")
            for l, ci in enumerate(cis):
                t0 = ci * TCH
                sl = slice(t0 * E, (t0 + TCH) * E)
                if ci % 2 == 0:
                    eng1, eng2 = nc.sync, nc.scalar
                else:
                    eng1, eng2 = nc.scalar, nc.sync
                w1t = w1p.tile([128, TCH * E], F16, tag="w1t")
                eng1.dma_start(w1t[:], w1m[:, sl])
                w2t = w2p.tile([128, TCH * E], F16, tag="w2t")
                eng2.dma_start(w2t[:], w2m[:, sl])
                w1ts.append(w1t)
                # diff in place of w2t (w2 is dead after this)
                nc.vector.tensor_tensor(w2t[:], w1t[:], w2t[:], op=Alu.subtract)
                nc.vector.tensor_reduce(
                    cost_w[:, l * TCH * NCAND : (l + 1) * TCH * NCAND].rearrange(
                        "p (t c) -> p t c", c=NCAND
                    ),
                    w2t[:].rearrange("p (t c k) -> p t c k", c=NCAND, k=K2),
                    axis=AX.X,
                    op=Alu.add,
                    apply_absolute_value=True,
                )
            return {"cis": cis, "w1ts": w1ts, "cost_w": cost_w}

        def part_chain_a(st):
            cis, cost_w = st["cis"], st["cost_w"]
            W = len(cis) * TCH
            cost3 = cost_w[:, : W * NCAND].rearrange("p (t c) -> p t c", c=NCAND)
            key_w = wp.tile([128, G * TCH * NCAND], F32, tag="key_w")
            key3 = key_w[:, : W * NCAND].rearrange("p (t c) -> p t c", c=NCAND)
            nc.vector.tensor_tensor(
                key3,
                cost3,
                rkey_t[:].unsqueeze(1).broadcast_to([128, W, NCAND]),
                op=Alu.add,
            )
            minkbm = wp.tile([128, G * TCH], F32, tag="minkbm")
            nc.vector.tensor_reduce(
                minkbm[:, :W], key3[:, :, 0:N2], axis=AX.X, op=Alu.min
            )
            mincbm = wp.tile([128, G * TCH], F32, tag="mincbm")
            nc.vector.tensor_reduce(
                mincbm[:, :W], cost3[:, :, 0:N2], axis=AX.X, op=Alu.min
            )
            kmv = key3[:, :, N2:NCAND].rearrange("p t one -> p (t one)")
            cmv = cost3[:, :, N2:NCAND].rearrange("p t one -> p (t one)")
            maskf = wp.tile([128, G * TCH], F32, tag="maskf")
            nc.vector.tensor_tensor(maskf[:, :W], kmv, minkbm[:, :W], op=Alu.is_lt)
            p0 = cis[0] * TCH
            nc.scalar.copy(st_msk[:, p0 : p0 + W], maskf[:, :W])
            nc.vector.tensor_tensor(
                st_mcv[:, p0 : p0 + W], cmv, mincbm[:, :W], op=Alu.min
            )
            frac = wp.tile([128, G * TCH], F32, tag="frac")
            nc.vector.tensor_tensor(
                frac[:, :W], minkbm[:, :W], mincbm[:, :W], op=Alu.subtract
            )
            rci = wp.tile([128, G * TCH], I16, tag="rci")
            nc.vector.tensor_scalar(
                rci[:, :W], frac[:, :W], 4096.0, None, op0=Alu.mult
            )
            gv = wp.tile([128, G * TCH * 32], F16, tag="gv")
            nc.gpsimd.ap_gather(
                gv[:, : W * 32],
                vecrk_t[:],
                rci[:, :W],
                channels=128,
                num_elems=NRANKC,
                d=2,
                num_idxs=16 * W,
            )
            st["maskf"] = maskf
            st["gv"] = gv
            st["p0"] = p0

        def part_chain_b(st):
            cis, w1ts = st["cis"], st["w1ts"]
            maskf, gv, p0 = st["maskf"], st["gv"], st["p0"]
            W = len(cis) * TCH
            vprod = wp.tile([128, G * TCH * 32], F16, tag="vprod")
            nc.vector.tensor_tensor(
                vprod[:, : W * 32].rearrange("p (t i d) -> p t i d", i=16, d=2),
                gv[:, : W * 32].rearrange("p (t i d) -> p t i d", i=16, d=2),
                imask_t[:]
                .rearrange("p (i d) -> p i d", d=2)
                .unsqueeze(1)
                .broadcast_to([128, W, 16, 2]),
                op=Alu.mult,
            )
            ovec = st_vec[:, p0 * 2 : (p0 + W) * 2].rearrange(
                "p (t d) -> p t d", d=2
            )
            with nc.allow_low_precision("one-hot extract, sums exact"):
                nc.vector.tensor_reduce(
                    ovec,
                    vprod[:, : W * 32].rearrange("p (t i d) -> p t d i", i=16, d=2),
                    axis=AX.X,
                    op=Alu.add,
                )
            vsl = st_vec[:, p0 * 2 : (p0 + W) * 2].rearrange("p (t d) -> p t d", d=2)
            v0 = vsl[:, :, 0:1].rearrange("p t one -> p (t one)")
            v1 = vsl[:, :, 1:2].rearrange("p t one -> p (t one)")
            # s = 7*v0 + v1; c = 24 - s; 49 - c = 25 + s
            s_w = wp.tile([128, G * TCH], F32, tag="s_w")
            nc.vector.scalar_tensor_tensor(
                s_w[:, :W], v0, 7.0, v1, op0=Alu.mult, op1=Alu.add
            )
            cidx = wp.tile([128, G * TCH], F32, tag="cidx")
            nc.vector.tensor_scalar(
                cidx[:, :W], s_w[:, :W], -1.0, 24.0, op0=Alu.mult, op1=Alu.add
            )
            t1 = wp.tile([128, G * TCH], F32, tag="t1")
            nc.vector.tensor_scalar(
                t1[:, :W], s_w[:, :W], 1.0, 25.0, op0=Alu.mult, op1=Alu.add
            )
            nc.vector.tensor_tensor(t1[:, :W], t1[:, :W], maskf[:, :W], op=Alu.mult)
            nc.vector.tensor_tensor(t1[:, :W], t1[:, :W], cidx[:, :W], op=Alu.add)
            for l, ci in enumerate(cis):
                # per-chunk tidx tile: ap_gather needs an aligned index base
                tidx = wp.tile([128, TCH], I16, tag=f"tidx{l}")
                nc.vector.tensor_tensor(
                    tidx[:],
                    t1[:, l * TCH : (l + 1) * TCH],
                    offs_t[:, 0:TCH],
                    op=Alu.add,
                )
                nc.gpsimd.ap_gather(
                    st_tplg[:, ci * (TCH * 16 * K2) : (ci + 1) * (TCH * 16 * K2)],
                    w1ts[l][:],
                    tidx[:],
                    channels=128,
                    num_elems=TCH * NCAND,
                    d=K2,
                    num_idxs=16 * TCH,
                )
            if not sim_safe:
                tpl_extract_dmas(cis[0], cis[-1] + 1)
            if cis[-1] + 1 == 12:
                flush_outputs(0, 12 * TCH)

        def flush_outputs(q0, q1):
            # staged vec/msk/mcv for per-partition pixel range [q0, q1)
            nc.sync.dma_start(
                o_vec[0:MAIN, :].rearrange("(p q) d -> p q d", p=128)[:, q0:q1],
                st_vec[:, q0 * 2 : q1 * 2],
            )
            nc.sync.dma_start(
                o_msk[0:MAIN, :].rearrange("(p q) one -> p q one", p=128)[:, q0:q1],
                st_msk[:, q0:q1],
            )
            nc.sync.dma_start(
                o_mcv[0:MAIN, :].rearrange("(p q) one -> p q one", p=128)[:, q0:q1],
                st_mcv[:, q0:q1],
            )

        def tpl_extract_dmas(c0, c1):
            # partition-strided extraction for chunks [c0, c1)
            o6 = o_tpl[0:MAIN, :].rearrange(
                "(q j c t) k -> q j c t k", j=16, c=NCHUNK, t=TCH
            )
            for j in range(16):
                sb = st_tplg[j::16].rearrange(
                    "q (c t i k) -> q c t i k", c=NCHUNK, t=TCH, i=16
                )[:, c0:c1, :, j : j + 1, :]
                eng = nc.sync if j % 2 == 0 else nc.scalar
                eng.dma_start(o6[:, j : j + 1, c0:c1], sb)

        # tail first so it overlaps the main stream: 128 pixels
        # [TAIL_OFF, PPC), one per partition
        tl_vec = sp.tile([128, 2], F16, tag="tl_vec")
        tl_tpl = sp.tile([128, K2], F16, tag="tl_tpl")
        tl_msk = sp.tile([128, 1], U8, tag="tl_msk")
        tl_mcv = sp.tile([128, 1], I32, tag="tl_mcv")
        chunk(
            1,
            w1[TAIL_OFF:PPC, :],
            w2[TAIL_OFF:PPC, :],
            tl_vec[:].rearrange("p (t d) -> p t d", t=1),
            tl_tpl[:].rearrange("p (t k) -> p t k", t=1),
            tl_msk[:],
            tl_mcv[:],
        )

        # staggered pipeline: group g's scalar chain + gathers are emitted
        # under group g+1's loads/subs/reduces
        groups = []
        _g = 0
        while _g < NCHUNK:
            n = G if NCHUNK - _g > 3 else (2 if NCHUNK - _g == 3 else NCHUNK - _g)
            groups.append(list(range(_g, _g + n)))
            _g += n
        prev = None
        for cis in groups:
            st = part_load(cis)
            if prev is not None:
                part_chain_b(prev)
            part_chain_a(st)
            prev = st
        part_chain_b(prev)

        flush_outputs(12 * TCH, TPART)
        # template extraction: partition p's valid gather column is i == p%16
        if sim_safe:
            # DVE one-hot extract into dense staging (CoreSim can't check the
            # partition-strided DMAs below)
            st_tpl = sp.tile([128, TPART * K2], F16, tag="st_tpl")
            imkk_t = cpool.tile([128, 16 * K2], F16, tag="imkk_t")
            nc.sync.dma_start(imkk_t[:], imaskk_d)
            for ci in range(NCHUNK):
                sl = st_tplg[:, ci * (TCH * 16 * K2) : (ci + 1) * (TCH * 16 * K2)]
                tp = wp.tile([128, TCH * 16 * K2], F16, tag="tp_ext")
                nc.vector.tensor_tensor(
                    tp[:].rearrange("p (t i k) -> p t i k", i=16, k=K2),
                    sl.rearrange("p (t i k) -> p t i k", i=16, k=K2),
                    imkk_t[:]
                    .rearrange("p (i k) -> p i k", k=K2)
                    .unsqueeze(1)
                    .broadcast_to([128, TCH, 16, K2]),
                    op=Alu.mult,
                )
                with nc.allow_low_precision("one-hot extract"):
                    nc.vector.tensor_reduce(
                        st_tpl[
                            :, ci * TCH * K2 : (ci + 1) * TCH * K2
                        ].rearrange("p (t k) -> p t k", k=K2),
                        tp[:].rearrange("p (t i k) -> p t k i", i=16, k=K2),
                        axis=AX.X,
                        op=Alu.add,
                    )
            nc.sync.dma_start(
                o_tpl[0:MAIN, :].rearrange("(p q) k -> p (q k)", p=128), st_tpl[:]
            )
        nc.sync.dma_start(o_vec[TAIL_OFF:PPC, :], tl_vec[:])
        nc.sync.dma_start(o_tpl[TAIL_OFF:PPC, :], tl_tpl[:])
        nc.sync.dma_start(o_msk[TAIL_OFF:PPC, :], tl_msk[:])
        nc.sync.dma_start(o_mcv[TAIL_OFF:PPC, :], tl_mcv[:])

    return nc


_CACHE = {}


def get_nc(sim_safe=False):
    key = ("nc", sim_safe)
    if key not in _CACHE:
        nc = bacc.Bacc("TRN2", target_bir_lowering=False, debug=False)
        build_program(nc, sim_safe=sim_safe)
        nc.compile()
        _CACHE[key] = nc
    return _CACHE[key]


def make_in_maps(w1, w2):
    w1 = np.ascontiguousarray(np.asarray(w1, dtype=np.float16).reshape(NPIX, E))
    w2 = np.ascontiguousarray(np.asarray(w2, dtype=np.float16).reshape(NPIX, E))
    rank_in = np.ascontiguousarray(np.broadcast_to(RANK64, (128, NCAND)))
    vec_in = np.ascontiguousarray(
        np.broadcast_to(VEC_DC.reshape(-1), (128, 2 * N2))
    )
    rkey_in = np.ascontiguousarray(np.broadcast_to(RKEY2, (128, NCAND)))
    vecrk_in = np.ascontiguousarray(
        np.broadcast_to(VEC_RANKC.reshape(-1), (128, NRANKC * 2))
    )
    imask_in = np.ascontiguousarray(IMASK2)
    offs_in = np.ascontiguousarray(np.broadcast_to(OFFS50, (128, TCH)))
    in_maps = []
    for c in range(NCORES):
        sl = slice(c * PPC, (c + 1) * PPC)
        in_maps.append(
            {
                "w1": np.ascontiguousarray(w1[sl]),
                "w2": np.ascontiguousarray(w2[sl]),
                "rank50": rank_in,
                "vecdc": vec_in,
                "rkey2": rkey_in,
                "vecrk": vecrk_in,
                "imask2": imask_in,
                "imaskk": np.ascontiguousarray(IMASKK),
                "offs50": offs_in,
            }
        )
    return in_maps


def assemble(results):
    vec = np.concatenate([results[c]["o_vec"] for c in range(NCORES)])
    tpl = np.concatenate([results[c]["o_tpl"] for c in range(NCORES)])
    msk = np.concatenate([results[c]["o_msk"] for c in range(NCORES)])
    mcv = np.concatenate([results[c]["o_mcv"] for c in range(NCORES)])
    return (
        vec.reshape(B, H, W, 2).astype(np.float16),
        tpl.reshape(B, H, W, 1, K2).astype(np.float16),
        msk.reshape(B, H, W, 1).astype(bool),
        mcv.reshape(B, H, W, 1).astype(np.int32),
    )


def kernel(w1, w2):
    nc = get_nc()
    in_maps = make_in_maps(w1, w2)
    res = run_bass_kernel_spmd(nc, in_maps, list(range(NCORES)))
    return assemble(res.results)


if __name__ == "__main__":
    rng = np.random.default_rng(0)
    w1 = rng.integers(0, 256, (B, H, W, NCAND, K2)).astype(np.float16)
    w2 = rng.integers(0, 256, (B, H, W, NCAND, K2)).astype(np.float16)
    outs = kernel(w1=w1, w2=w2)
    for o in outs:
        print(o.shape, o.dtype)
